# revision 6
# baseline (speedup 1.0000x reference)
"""GAT 2-layer kernel for 8 Trainium2 NeuronCores (Bass/Tile).

Strategy (1D partition by dst):
- Each core owns N/8 dst nodes (padded to blocks of 128). Host permutes each
  core's nodes so in-degrees are sorted descending -> per-block edge-slot
  capacity cap_b (= cross-core max block degree) stays tight.
- Transform phase: per node-block matmul x @ [W | W@a_src | W@a_dst] gives
  [Wh | es | ed] in one pass. [Wh|es] rows (bf16, 128 elems = 256B) form the
  gather table; ed stays on-chip (per-partition, dst-local).
- AllGather the per-core tables -> every core holds the full [N, 128] bf16
  table in DRAM.
- Edge phase: per dst-block one dma_gather with PAIR rows (elem 256 bf16 =
  512B covering nodes 2k/2k+1, idx = src>>1 fits int16); edge slot (p, j) =
  j-th in-edge of the block's p-th dst. Half-select is folded into the
  attention scaling (exA = ex*(1-m), exB = ex*m).
- Attention: e = lrelu(es_sel + ed), ex = exp(e) (max-subtraction is skipped:
  |e| is bounded by a few units so exp cannot overflow; softmax is shift
  invariant). numer = sum_j ex*Wh via one multiply + one strided reduce;
  den = sum_j ex. Pad slots point at a pad pair-row whose es = -1e30 -> ex=0.
- Layer 2 reuses the same edge slots/indices; final row softmax on chip.
"""

import sys, os
sys.path.insert(0, '/opt/trn_rl_repo')

import numpy as np

# ---- problem constants (from the reference; hardcoded, not read from disk) ----
N = 50000
E = 800000
F_IN = 512
H1 = 8
F_HID = 8
OUT1 = H1 * F_HID          # 64
H2 = 1
NLAB = 64
LRELU_SLOPE = 0.2
NCORES = 8
P = 128

N_PC = N // NCORES         # 6250
NBLK = (N_PC + P - 1) // P # 49
N_PAD = NBLK * P           # 6272
N_TOT = N_PAD * NCORES     # 50176
NPAIR = N_TOT // 2         # 25088
ROW = 128                  # table row elems (bf16): [Wh 64 | es 8 | pad]
ES_OFF = 64
NEG_BIG = -1.0e30
MAX_GATHER = 8192          # HW limit for one dma_gather


def _bf16(x):
    import jax.numpy as jnp
    return np.asarray(jnp.asarray(np.asarray(x), dtype=jnp.bfloat16))


def host_prep(inputs, W1, a1_src, a1_dst, W2, a2_src, a2_dst, src, dst):
    """Pure-numpy preparation of all per-core tensors + layout metadata."""
    inputs = np.asarray(inputs); src = np.asarray(src); dst = np.asarray(dst)
    W1 = np.asarray(W1); W2 = np.asarray(W2)
    a1_src = np.asarray(a1_src); a1_dst = np.asarray(a1_dst)
    a2_src = np.asarray(a2_src); a2_dst = np.asarray(a2_dst)

    core_of = dst // N_PC                      # owner core per edge
    # per-core node permutation: sort own nodes by in-degree desc (pads last)
    perms = []          # perms[c][i] = original node id at permuted position i
    inv_pos = np.zeros(N, dtype=np.int64)      # node -> position within its core
    degs = np.bincount(dst, minlength=N)
    for c in range(NCORES):
        own = np.arange(c * N_PC, (c + 1) * N_PC)
        order = own[np.argsort(-degs[own], kind='stable')]
        perm = np.concatenate([order, np.full(N_PAD - N_PC, -1, dtype=np.int64)])
        perms.append(perm)
        inv_pos[order] = np.arange(N_PC)

    # global table row of node n (pad rows belong to tail positions)
    def table_row(n):
        return (n // N_PC) * N_PAD + inv_pos[n]

    # per-core per-block caps (cross-core max) ------------------------------
    deg_mat = np.zeros((NCORES, N_PAD), dtype=np.int64)
    for c in range(NCORES):
        real = perms[c] >= 0
        deg_mat[c, real] = degs[perms[c][real]]
    caps = deg_mat.reshape(NCORES, NBLK, P).max(axis=2).max(axis=0)  # [NBLK]
    caps = np.maximum(caps, 1)
    slot_off = np.concatenate([[0], np.cumsum(caps)])                # col offsets
    tot_cols = int(slot_off[-1])

    # pad pair-row: last two table rows are core-7 pad nodes (deg 0, x rows 0)
    PAD_PAIR = (N_TOT - 2) // 2

    # per-core edge slot assignment -----------------------------------------
    idx_all = np.full((NCORES, P, tot_cols), PAD_PAIR, dtype=np.int64)
    m_all = np.zeros((NCORES, P, tot_cols), dtype=np.float32)
    tr_src = table_row(src)
    pos_in_core = inv_pos[dst]                 # permuted position of dst
    for c in range(NCORES):
        sel = core_of == c
        s_rows = tr_src[sel]
        d_pos = pos_in_core[sel]
        order = np.argsort(d_pos, kind='stable')
        s_rows = s_rows[order]; d_pos = d_pos[order]
        # j-th edge of each dst
        jj = np.arange(len(d_pos)) - np.searchsorted(d_pos, d_pos, side='left')
        b = d_pos // P; pp = d_pos % P
        cols = slot_off[b] + jj
        assert (jj < caps[b]).all()
        idx_all[c, pp, cols] = s_rows // 2
        m_all[c, pp, cols] = (s_rows % 2).astype(np.float32)

    # wrapped int16 index layout per block: flat order i=(col*128+p),
    # reshape(-1,16).T, tiled 8x over partitions
    idx_wrapped = np.zeros((NCORES, P, tot_cols * 8), dtype=np.int16)
    for c in range(NCORES):
        for b in range(NBLK):
            o0, o1 = slot_off[b], slot_off[b + 1]
            flat = idx_all[c][:, o0:o1].T.reshape(-1)      # (col, p) order
            wr = np.tile(flat.reshape(-1, 16).T, (8, 1)).astype(np.int16)
            idx_wrapped[c][:, o0 * 8:o1 * 8] = wr

    # weights ---------------------------------------------------------------
    W1cat = W1.transpose(1, 0, 2).reshape(F_IN, OUT1)      # [512, 64]
    ws1 = np.einsum('hfo,ho->fh', W1, a1_src)              # [512, 8]
    wd1 = np.einsum('hfo,ho->fh', W1, a1_dst)              # [512, 8]
    W1full = np.concatenate([W1cat, ws1, wd1], axis=1)     # [512, 80]
    W2cat = W2.transpose(1, 0, 2).reshape(OUT1, NLAB)      # [64, 64]
    ws2 = np.einsum('hfo,ho->fh', W2, a2_src)              # [64, 1]
    wd2 = np.einsum('hfo,ho->fh', W2, a2_dst)              # [64, 1]
    W2full = np.concatenate([W2cat, ws2, wd2], axis=1)     # [64, 66]

    # per-core transposed inputs (permuted, padded) -------------------------
    xT = np.zeros((NCORES, F_IN, N_PAD), dtype=np.float32)
    for c in range(NCORES):
        real = perms[c] >= 0
        xT[c][:, real] = inputs[perms[c][real]].T

    meta = dict(caps=caps, slot_off=slot_off, tot_cols=tot_cols, perms=perms)
    per_core = dict(
        xT=[_bf16(xT[c]) for c in range(NCORES)],
        idx=[idx_wrapped[c] for c in range(NCORES)],
        m=[_bf16(m_all[c]) for c in range(NCORES)],
    )
    shared = dict(W1full=_bf16(W1full), W2full=_bf16(W2full))
    return meta, per_core, shared


def build_kernel(meta):
    import concourse.bass as bass
    import concourse.bacc as bacc
    import concourse.tile as tile
    from concourse import mybir
    from concourse.masks import make_identity

    bf16 = mybir.dt.bfloat16; f32 = mybir.dt.float32; i16 = mybir.dt.int16
    AL = mybir.AluOpType; AF = mybir.ActivationFunctionType; AX = mybir.AxisListType

    caps = [int(x) for x in meta['caps']]
    slot_off = [int(x) for x in meta['slot_off']]
    TC = int(meta['tot_cols'])

    nc = bacc.Bacc("TRN2", target_bir_lowering=False, debug=False,
                   enable_asserts=True, num_devices=NCORES)

    t_xT = nc.dram_tensor("xT", [F_IN, N_PAD], bf16, kind="ExternalInput").ap()
    t_idx = nc.dram_tensor("idx", [P, TC * 8], i16, kind="ExternalInput").ap()
    t_m = nc.dram_tensor("m", [P, TC], bf16, kind="ExternalInput").ap()
    t_W1 = nc.dram_tensor("W1full", [F_IN, 80], bf16, kind="ExternalInput").ap()
    t_W2 = nc.dram_tensor("W2full", [OUT1, 66], bf16, kind="ExternalInput").ap()
    t_out = nc.dram_tensor("out", [N_PAD, NLAB], f32, kind="ExternalOutput").ap()

    KCH = F_IN // P  # 4 k-chunks

    with tile.TileContext(nc) as tc:
        with tc.tile_pool(name="dram", bufs=1, space="DRAM") as dram, \
             tc.tile_pool(name="const", bufs=1) as cpool, \
             tc.tile_pool(name="work", bufs=3) as wpool, \
             tc.tile_pool(name="gath", bufs=2) as gpool, \
             tc.tile_pool(name="msgsp", bufs=2) as mpool, \
             tc.tile_pool(name="psum", bufs=2, space="PSUM") as pp, \
             tc.tile_pool(name="psum1", bufs=2, space="PSUM") as pp1:

            ident = cpool.tile([P, P], f32)
            make_identity(nc, ident[:])

            w1_sb = cpool.tile([P, KCH, 80], bf16)
            nc.sync.dma_start(out=w1_sb[:], in_=t_W1.rearrange("(k p) w -> p k w", p=P))
            w2_sb = cpool.tile([OUT1, 66], bf16)
            nc.sync.dma_start(out=w2_sb[:], in_=t_W2[:])
            m_sb = cpool.tile([P, TC], bf16)
            nc.sync.dma_start(out=m_sb[:], in_=t_m[:])
            idx_sb = cpool.tile([P, TC * 8], i16)
            nc.sync.dma_start(out=idx_sb[:], in_=t_idx[:])

            ed1_all = cpool.tile([P, NBLK, H1], f32)
            ed2_all = cpool.tile([P, NBLK, 1], f32)

            # DRAM tables
            T1_loc = dram.tile([N_PAD, ROW], bf16)
            T2_loc = dram.tile([N_PAD, ROW], bf16)
            T1_full = dram.tile([N_TOT, ROW], bf16, addr_space="Shared")
            T2_full = dram.tile([N_TOT, ROW], bf16, addr_space="Shared")

            # ---------------- phase 1: transform layer 1 ----------------
            for b in range(NBLK):
                ps = pp.tile([P, 80], f32, tag="tf1", space="PSUM")
                for k in range(KCH):
                    xt = wpool.tile([P, P], bf16, tag="xt", bufs=4)
                    nc.sync.dma_start(out=xt[:],
                                      in_=t_xT[k * P:(k + 1) * P, b * P:(b + 1) * P])
                    nc.tensor.matmul(out=ps[:], lhsT=xt[:],
                                     rhs=w1_sb[:, k, :], start=(k == 0), stop=(k == KCH - 1))
                row = wpool.tile([P, ROW], bf16, tag="trow")
                nc.vector.memset(row[:], 0.0)
                nc.vector.tensor_copy(out=row[:, 0:72], in_=ps[:, 0:72])
                if b == NBLK - 1:
                    # pad nodes (tail partitions) must have es = -1e30
                    npad = N_PAD - N_PC  # 22
                    nc.gpsimd.affine_select(
                        out=row[:, ES_OFF:ES_OFF + H1], in_=row[:, ES_OFF:ES_OFF + H1],
                        pattern=[[0, H1]], compare_op=mybir.AluOpType.is_ge,
                        fill=NEG_BIG, base=P - npad - 1, channel_multiplier=-1)
                nc.sync.dma_start(out=T1_loc[b * P:(b + 1) * P, :], in_=row[:])
                nc.vector.tensor_copy(out=ed1_all[:, b, :], in_=ps[:, 72:80])

            # ---------------- all-gather table 1 ----------------
            nc.gpsimd.collective_compute(
                "AllGather", mybir.AluOpType.bypass,
                replica_groups=[list(range(NCORES))],
                ins=[T1_loc[:].opt()], outs=[T1_full[:].opt()])

            # pair view of the full table: [NPAIR, 256]
            T1_pair = T1_full[:].rearrange("(q t) r -> q (t r)", t=2)
            T2_pair = T2_full[:].rearrange("(q t) r -> q (t r)", t=2)

            def edge_block(b, T_pair, ed_ap, H, layer):
                """Process dst-block b for one layer. Returns (numer, den) tiles."""
                cap = caps[b]; o0 = slot_off[b]
                ni = cap * P
                g = gpool.tile([P, caps[0], 256], bf16, tag="g")
                n_g = min((ni + MAX_GATHER - 1) // MAX_GATHER, 8)
                step = ((cap + n_g - 1) // n_g)
                c0 = 0
                while c0 < cap:
                    c1 = min(c0 + step, cap)
                    nc.gpsimd.dma_gather(
                        out_ap=g[:, c0:c1, :], in_ap=T_pair,
                        idxs_ap=idx_sb[:, (o0 + c0) * 8:(o0 + c1) * 8],
                        num_idxs=(c1 - c0) * P, num_idxs_reg=(c1 - c0) * P,
                        elem_size=256, single_packet=False)
                    c0 = c1
                mm = m_sb[:, o0:o0 + cap]

                # es select + ed + lrelu + exp
                esA = g[:, 0:cap, ES_OFF:ES_OFF + H]
                esB = g[:, 0:cap, 128 + ES_OFF:128 + ES_OFF + H]
                d1 = wpool.tile([P, caps[0], H1], f32, tag="d1", name="d1t")[:, 0:cap, 0:H]
                nc.vector.tensor_tensor(out=d1, in0=esB, in1=esA, op=AL.subtract)
                t1 = wpool.tile([P, caps[0], H1], f32, tag="tt1", name="tt1t")[:, 0:cap, 0:H]
                nc.vector.tensor_tensor(out=t1, in0=d1,
                                        in1=mm.unsqueeze(-1).broadcast_to([P, cap, H]),
                                        op=AL.mult)
                s = wpool.tile([P, caps[0], H1], f32, tag="s", name="st")[:, 0:cap, 0:H]
                nc.vector.tensor_tensor(out=s, in0=esA, in1=t1, op=AL.add)
                nc.vector.tensor_tensor(out=s, in0=s,
                                        in1=ed_ap.unsqueeze(1).broadcast_to([P, cap, H]),
                                        op=AL.add)
                e = wpool.tile([P, caps[0], H1], f32, tag="e", name="et")[:, 0:cap, 0:H]
                nc.scalar.activation(out=e, in_=s, func=AF.Lrelu, alpha=LRELU_SLOPE)
                ex = wpool.tile([P, caps[0], H1], f32, tag="exx", name="exxt")[:, 0:cap, 0:H]
                nc.scalar.activation(out=ex, in_=e, func=AF.Exp)

                # exps[p, j, half, h]: exB' = ex*m, exA' = ex - exB'
                exps = wpool.tile([P, caps[0], 2, H1], bf16, tag="exps", name="expst")[:, 0:cap, :, 0:H]
                nc.vector.tensor_tensor(out=exps[:, :, 1, :], in0=ex,
                                        in1=mm.unsqueeze(-1).broadcast_to([P, cap, H]),
                                        op=AL.mult)
                nc.vector.tensor_tensor(out=exps[:, :, 0, :], in0=ex, in1=exps[:, :, 1, :],
                                        op=AL.subtract)

                # msgs[p, j, half, h, o] = Wh[p, j, half, h, o] * exps[p, j, half, h]
                OUTD = 64
                msgs = mpool.tile([P, caps[0], 2, OUTD], f32, tag="msgs", name="msgst")[:, 0:cap, :, :]
                wh = g[:, 0:cap, :].rearrange("p j (t r) -> p j t r", t=2)[:, :, :, 0:OUTD]
                if H > 1:
                    wh5 = wh.rearrange("p j t (h o) -> p j t h o", h=H)
                    ex5 = exps.unsqueeze(-1).broadcast_to([P, cap, 2, H, OUTD // H])
                    nc.vector.tensor_tensor(out=msgs.rearrange("p j t (h o) -> p j t h o", h=H),
                                            in0=wh5, in1=ex5, op=AL.mult)
                else:
                    ex4 = exps.broadcast_to([P, cap, 2, OUTD])
                    nc.vector.tensor_tensor(out=msgs, in0=wh, in1=ex4, op=AL.mult)

                numer = wpool.tile([P, OUTD], f32, tag="num")
                # reduce over (j, half) keeping (h, o): innermost = combined (j,t)
                nc.vector.tensor_reduce(
                    out=numer[:], in_=msgs.rearrange("p j t r -> p r (j t)"),
                    axis=AX.X, op=AL.add)
                den = wpool.tile([P, H1], f32, tag="den", name="dent")[:, 0:H]
                nc.vector.tensor_reduce(out=den, in_=exps.rearrange("p j t h -> p h (j t)"),
                                        axis=AX.X, op=AL.add)
                nc.vector.tensor_scalar_add(out=den, in0=den, scalar1=1e-10)
                rec = wpool.tile([P, H1], f32, tag="rec", name="rect")[:, 0:H]
                nc.vector.reciprocal(out=rec, in_=den)
                hpre = wpool.tile([P, OUTD], f32, tag="hpre")
                if H > 1:
                    nc.vector.tensor_tensor(
                        out=hpre[:].rearrange("p (h o) -> p h o", h=H),
                        in0=numer[:].rearrange("p (h o) -> p h o", h=H),
                        in1=rec.unsqueeze(-1).broadcast_to([P, H, OUTD // H]), op=AL.mult)
                else:
                    nc.vector.tensor_tensor(out=hpre[:], in0=numer[:],
                                            in1=rec.broadcast_to([P, OUTD]), op=AL.mult)
                return hpre

            # ---------------- phase 2: layer-1 edges + layer-2 transform ----------------
            for b in range(NBLK):
                hpre = edge_block(b, T1_pair, ed1_all[:, b, :], H1, 1)
                # ELU: h = relu(x) + min(exp(x),1) - 1
                ex_h = wpool.tile([P, OUT1], f32, tag="eluex")
                nc.scalar.activation(out=ex_h[:], in_=hpre[:], func=AF.Exp)
                nc.vector.tensor_scalar_min(out=ex_h[:], in0=ex_h[:], scalar1=1.0)
                r_h = wpool.tile([P, OUT1], f32, tag="elur")
                nc.vector.tensor_scalar_max(out=r_h[:], in0=hpre[:], scalar1=0.0)
                h = wpool.tile([P, OUT1], f32, tag="hfin")
                nc.vector.tensor_tensor(out=h[:], in0=r_h[:], in1=ex_h[:], op=AL.add)
                nc.vector.tensor_scalar_add(out=h[:], in0=h[:], scalar1=-1.0)
                # transpose h -> [64, 128]
                hT_ps = pp1.tile([OUT1, P], f32, tag="hT", space="PSUM")
                nc.tensor.transpose(out=hT_ps[:], in_=h[:], identity=ident[:])
                hT = wpool.tile([OUT1, P], bf16, tag="hTb")
                nc.vector.tensor_copy(out=hT[:], in_=hT_ps[:])
                # layer-2 transform
                ps2 = pp.tile([P, 66], f32, tag="tf2", space="PSUM")
                nc.tensor.matmul(out=ps2[:], lhsT=hT[:], rhs=w2_sb[:], start=True, stop=True)
                row2 = wpool.tile([P, ROW], bf16, tag="trow")
                nc.vector.memset(row2[:], 0.0)
                nc.vector.tensor_copy(out=row2[:, 0:65], in_=ps2[:, 0:65])
                if b == NBLK - 1:
                    npad = N_PAD - N_PC
                    nc.gpsimd.affine_select(
                        out=row2[:, ES_OFF:ES_OFF + 1], in_=row2[:, ES_OFF:ES_OFF + 1],
                        pattern=[[0, 1]], compare_op=mybir.AluOpType.is_ge,
                        fill=NEG_BIG, base=P - npad - 1, channel_multiplier=-1)
                nc.sync.dma_start(out=T2_loc[b * P:(b + 1) * P, :], in_=row2[:])
                nc.vector.tensor_copy(out=ed2_all[:, b, :], in_=ps2[:, 65:66])

            # ---------------- all-gather table 2 ----------------
            nc.gpsimd.collective_compute(
                "AllGather", mybir.AluOpType.bypass,
                replica_groups=[list(range(NCORES))],
                ins=[T2_loc[:].opt()], outs=[T2_full[:].opt()])

            # ---------------- phase 3: layer-2 edges + softmax ----------------
            for b in range(NBLK):
                opre = edge_block(b, T2_pair, ed2_all[:, b, :], H2, 2)
                rm = wpool.tile([P, 1], f32, tag="rm")
                nc.vector.tensor_reduce(out=rm[:], in_=opre[:], axis=AX.X,
                                        op=AL.max, negate=True)
                z = wpool.tile([P, NLAB], f32, tag="z")
                zsum = wpool.tile([P, 1], f32, tag="zsum")
                nc.scalar.activation(out=z[:], in_=opre[:], func=AF.Exp,
                                     bias=rm[:], accum_out=zsum[:])
                recs = wpool.tile([P, 1], f32, tag="recs")
                nc.vector.reciprocal(out=recs[:], in_=zsum[:])
                fin = wpool.tile([P, NLAB], f32, tag="fin")
                nc.vector.tensor_tensor(out=fin[:], in0=z[:],
                                        in1=recs[:].broadcast_to([P, NLAB]), op=AL.mult)
                nc.sync.dma_start(out=t_out[b * P:(b + 1) * P, :], in_=fin[:])

    nc.compile()
    return nc


def kernel(inputs, W1, a1_src, a1_dst, W2, a2_src, a2_dst, src, dst):
    from concourse import bass_utils
    meta, per_core, shared = host_prep(inputs, W1, a1_src, a1_dst, W2,
                                       a2_src, a2_dst, src, dst)
    nc = build_kernel(meta)
    in_maps = []
    for c in range(NCORES):
        in_maps.append(dict(
            xT=per_core['xT'][c], idx=per_core['idx'][c], m=per_core['m'][c],
            W1full=shared['W1full'], W2full=shared['W2full']))
    res = bass_utils.run_bass_kernel_spmd(
        nc, in_maps, core_ids=list(range(NCORES)),
        trace=bool(int(os.environ.get("GAT_TRACE", "0"))),
        trace_cores=list(range(NCORES)) if int(os.environ.get("GAT_TRACE", "0")) else None)
    kernel.last_exec_time_ns = res.exec_time_ns
    out = np.zeros((N, NLAB), dtype=np.float32)
    for c in range(NCORES):
        o = res.results[c]["out"]
        perm = meta['perms'][c]
        real = perm >= 0
        out[perm[real]] = o[real]
    return out


def mirror(inputs, W1, a1_src, a1_dst, W2, a2_src, a2_dst, src, dst):
    """Numpy mirror of the kernel's exact dataflow (incl. bf16 rounding of
    tables) for layout validation without hardware."""
    meta, per_core, shared = host_prep(inputs, W1, a1_src, a1_dst, W2,
                                       a2_src, a2_dst, src, dst)
    caps = meta['caps']; slot_off = meta['slot_off']; TC = meta['tot_cols']
    W1full = shared['W1full'].astype(np.float32)
    W2full = shared['W2full'].astype(np.float32)
    out = np.zeros((N, NLAB), dtype=np.float32)

    # build tables per core, then allgather
    T1 = np.zeros((N_TOT, ROW), dtype=np.float32)
    ed1 = np.zeros((NCORES, N_PAD, H1), dtype=np.float32)
    for c in range(NCORES):
        xT = per_core['xT'][c].astype(np.float32)
        t = xT.T @ W1full                       # [N_PAD, 80]
        rows = np.zeros((N_PAD, ROW), np.float32)
        rows[:, 0:72] = t[:, 0:72]
        rows[N_PC:, ES_OFF:ES_OFF + H1] = NEG_BIG
        T1[c * N_PAD:(c + 1) * N_PAD] = _bf16(rows).astype(np.float32)
        ed1[c] = t[:, 72:80]

    def edge_phase(c, T, ed, H):
        Tp = T.reshape(NPAIR, 256)
        idx = per_core['idx'][c]
        m = per_core['m'][c].astype(np.float32)
        res = np.zeros((N_PAD, OUT1), np.float32)
        for b in range(NBLK):
            cap = caps[b]; o0 = slot_off[b]
            # unwrap idx: stored wrapped per block
            wr = idx[:16, o0 * 8:(o0 + cap) * 8]
            flat = wr.T.reshape(-1)             # undo .reshape(-1,16).T
            g = Tp[flat.astype(np.int64)].reshape(cap, P, 256).transpose(1, 0, 2)
            mm = m[:, o0:o0 + cap]
            esA = g[:, :, ES_OFF:ES_OFF + H]
            esB = g[:, :, 128 + ES_OFF:128 + ES_OFF + H]
            es = esA + mm[:, :, None] * (esB - esA)
            s = es + ed[b * P:(b + 1) * P].reshape(P, 1, H)
            e = np.where(s > 0, s, LRELU_SLOPE * s)
            ex = np.exp(e)
            exB = ex * mm[:, :, None]; exA = ex - exB
            whA = g[:, :, 0:64]; whB = g[:, :, 128:192]
            if H > 1:
                o = OUT1 // H
                msA = whA.reshape(P, cap, H, o) * exA[:, :, :, None]
                msB = whB.reshape(P, cap, H, o) * exB[:, :, :, None]
                numer = (msA + msB).sum(axis=1).reshape(P, OUT1)
                den = (exA + exB).sum(axis=1)
                hpre = (numer.reshape(P, H, o) / (den[:, :, None] + 1e-10)).reshape(P, OUT1)
            else:
                msA = whA * exA; msB = whB * exB
                numer = (msA + msB).sum(axis=1)
                den = (exA + exB).sum(axis=1)
                hpre = numer / (den + 1e-10)
            res[b * P:(b + 1) * P] = hpre
        return res

    T2 = np.zeros((N_TOT, ROW), dtype=np.float32)
    ed2 = np.zeros((NCORES, N_PAD, 1), dtype=np.float32)
    h_all = {}
    for c in range(NCORES):
        hpre = edge_phase(c, T1, ed1[c], H1)
        h = np.maximum(hpre, 0) + np.minimum(np.exp(hpre), 1.0) - 1.0
        h_all[c] = h
        t2 = _bf16(h).astype(np.float32) @ W2full
        rows = np.zeros((N_PAD, ROW), np.float32)
        rows[:, 0:65] = t2[:, 0:65]
        rows[N_PC:, ES_OFF:ES_OFF + 1] = NEG_BIG
        T2[c * N_PAD:(c + 1) * N_PAD] = _bf16(rows).astype(np.float32)
        ed2[c] = t2[:, 65:66]

    for c in range(NCORES):
        opre = edge_phase(c, T2, ed2[c], H2)[:, 0:NLAB]
        z = np.exp(opre - opre.max(axis=1, keepdims=True))
        fin = z / z.sum(axis=1, keepdims=True)
        perm = meta['perms'][c]; real = perm >= 0
        out[perm[real]] = fin[real]
    return out


# revision 7
# speedup vs baseline: 1.0040x; 1.0040x over previous
"""GAT 2-layer kernel for 8 Trainium2 NeuronCores (Bass/Tile).

Strategy (1D partition by dst):
- Each core owns N/8 dst nodes (padded to blocks of 128). Host permutes each
  core's nodes so in-degrees are sorted descending -> per-block edge-slot
  capacity cap_b (= cross-core max block degree) stays tight.
- Transform phase: per node-block matmul x @ [W | W@a_src | W@a_dst] gives
  [Wh | es | ed] in one pass. [Wh|es] rows (bf16, 128 elems = 256B) form the
  gather table; ed stays on-chip (per-partition, dst-local).
- AllGather the per-core tables -> every core holds the full [N, 128] bf16
  table in DRAM.
- Edge phase: per dst-block one dma_gather with PAIR rows (elem 256 bf16 =
  512B covering nodes 2k/2k+1, idx = src>>1 fits int16); edge slot (p, j) =
  j-th in-edge of the block's p-th dst. Half-select is folded into the
  attention scaling (exA = ex*(1-m), exB = ex*m).
- Attention: e = lrelu(es_sel + ed), ex = exp(e) (max-subtraction is skipped:
  |e| is bounded by a few units so exp cannot overflow; softmax is shift
  invariant). numer = sum_j ex*Wh via one multiply + one strided reduce;
  den = sum_j ex. Pad slots point at a pad pair-row whose es = -1e30 -> ex=0.
- Layer 2 reuses the same edge slots/indices; final row softmax on chip.
"""

import sys, os
sys.path.insert(0, '/opt/trn_rl_repo')

import numpy as np

# ---- problem constants (from the reference; hardcoded, not read from disk) ----
N = 50000
E = 800000
F_IN = 512
H1 = 8
F_HID = 8
OUT1 = H1 * F_HID          # 64
H2 = 1
NLAB = 64
LRELU_SLOPE = 0.2
NCORES = 8
P = 128

N_PC = N // NCORES         # 6250
NBLK = (N_PC + P - 1) // P # 49
N_PAD = NBLK * P           # 6272
N_TOT = N_PAD * NCORES     # 50176
NPAIR = N_TOT // 2         # 25088
ROW = 128                  # table row elems (bf16): [Wh 64 | es 8 | pad]
ES_OFF = 64
NEG_BIG = -1.0e30
MAX_GATHER = 8192          # HW limit for one dma_gather


def _bf16(x):
    import jax.numpy as jnp
    return np.asarray(jnp.asarray(np.asarray(x), dtype=jnp.bfloat16))


def host_prep(inputs, W1, a1_src, a1_dst, W2, a2_src, a2_dst, src, dst):
    """Pure-numpy preparation of all per-core tensors + layout metadata."""
    inputs = np.asarray(inputs); src = np.asarray(src); dst = np.asarray(dst)
    W1 = np.asarray(W1); W2 = np.asarray(W2)
    a1_src = np.asarray(a1_src); a1_dst = np.asarray(a1_dst)
    a2_src = np.asarray(a2_src); a2_dst = np.asarray(a2_dst)

    core_of = dst // N_PC                      # owner core per edge
    # per-core node permutation: sort own nodes by in-degree desc (pads last)
    perms = []          # perms[c][i] = original node id at permuted position i
    inv_pos = np.zeros(N, dtype=np.int64)      # node -> position within its core
    degs = np.bincount(dst, minlength=N)
    for c in range(NCORES):
        own = np.arange(c * N_PC, (c + 1) * N_PC)
        order = own[np.argsort(-degs[own], kind='stable')]
        perm = np.concatenate([order, np.full(N_PAD - N_PC, -1, dtype=np.int64)])
        perms.append(perm)
        inv_pos[order] = np.arange(N_PC)

    # global table row of node n (pad rows belong to tail positions)
    def table_row(n):
        return (n // N_PC) * N_PAD + inv_pos[n]

    # per-core per-block caps (cross-core max) ------------------------------
    deg_mat = np.zeros((NCORES, N_PAD), dtype=np.int64)
    for c in range(NCORES):
        real = perms[c] >= 0
        deg_mat[c, real] = degs[perms[c][real]]
    caps = deg_mat.reshape(NCORES, NBLK, P).max(axis=2).max(axis=0)  # [NBLK]
    caps = np.maximum(caps, 1)
    slot_off = np.concatenate([[0], np.cumsum(caps)])                # col offsets
    tot_cols = int(slot_off[-1])

    # pad pair-row: last two table rows are core-7 pad nodes (deg 0, x rows 0)
    PAD_PAIR = (N_TOT - 2) // 2

    # per-core edge slot assignment -----------------------------------------
    idx_all = np.full((NCORES, P, tot_cols), PAD_PAIR, dtype=np.int64)
    m_all = np.zeros((NCORES, P, tot_cols), dtype=np.float32)
    tr_src = table_row(src)
    pos_in_core = inv_pos[dst]                 # permuted position of dst
    for c in range(NCORES):
        sel = core_of == c
        s_rows = tr_src[sel]
        d_pos = pos_in_core[sel]
        order = np.argsort(d_pos, kind='stable')
        s_rows = s_rows[order]; d_pos = d_pos[order]
        # j-th edge of each dst
        jj = np.arange(len(d_pos)) - np.searchsorted(d_pos, d_pos, side='left')
        b = d_pos // P; pp = d_pos % P
        cols = slot_off[b] + jj
        assert (jj < caps[b]).all()
        idx_all[c, pp, cols] = s_rows // 2
        m_all[c, pp, cols] = (s_rows % 2).astype(np.float32)

    # wrapped int16 index layout per block: flat order i=(col*128+p),
    # reshape(-1,16).T, tiled 8x over partitions
    idx_wrapped = np.zeros((NCORES, P, tot_cols * 8), dtype=np.int16)
    for c in range(NCORES):
        for b in range(NBLK):
            o0, o1 = slot_off[b], slot_off[b + 1]
            flat = idx_all[c][:, o0:o1].T.reshape(-1)      # (col, p) order
            wr = np.tile(flat.reshape(-1, 16).T, (8, 1)).astype(np.int16)
            idx_wrapped[c][:, o0 * 8:o1 * 8] = wr

    # weights ---------------------------------------------------------------
    W1cat = W1.transpose(1, 0, 2).reshape(F_IN, OUT1)      # [512, 64]
    ws1 = np.einsum('hfo,ho->fh', W1, a1_src)              # [512, 8]
    wd1 = np.einsum('hfo,ho->fh', W1, a1_dst)              # [512, 8]
    W1full = np.concatenate([W1cat, ws1, wd1], axis=1)     # [512, 80]
    W2cat = W2.transpose(1, 0, 2).reshape(OUT1, NLAB)      # [64, 64]
    ws2 = np.einsum('hfo,ho->fh', W2, a2_src)              # [64, 1]
    wd2 = np.einsum('hfo,ho->fh', W2, a2_dst)              # [64, 1]
    W2full = np.concatenate([W2cat, ws2, wd2], axis=1)     # [64, 66]

    # per-core transposed inputs (permuted, padded) -------------------------
    xT = np.zeros((NCORES, F_IN, N_PAD), dtype=np.float32)
    for c in range(NCORES):
        real = perms[c] >= 0
        xT[c][:, real] = inputs[perms[c][real]].T

    meta = dict(caps=caps, slot_off=slot_off, tot_cols=tot_cols, perms=perms)
    per_core = dict(
        xT=[_bf16(xT[c]) for c in range(NCORES)],
        idx=[idx_wrapped[c] for c in range(NCORES)],
        m=[_bf16(m_all[c]) for c in range(NCORES)],
    )
    shared = dict(W1full=_bf16(W1full), W2full=_bf16(W2full))
    return meta, per_core, shared


def build_kernel(meta):
    import concourse.bass as bass
    import concourse.bacc as bacc
    import concourse.tile as tile
    from concourse import mybir
    from concourse.masks import make_identity

    bf16 = mybir.dt.bfloat16; f32 = mybir.dt.float32; i16 = mybir.dt.int16
    AL = mybir.AluOpType; AF = mybir.ActivationFunctionType; AX = mybir.AxisListType

    caps = [int(x) for x in meta['caps']]
    slot_off = [int(x) for x in meta['slot_off']]
    TC = int(meta['tot_cols'])

    nc = bacc.Bacc("TRN2", target_bir_lowering=False, debug=False,
                   enable_asserts=True, num_devices=NCORES)

    t_xT = nc.dram_tensor("xT", [F_IN, N_PAD], bf16, kind="ExternalInput").ap()
    t_idx = nc.dram_tensor("idx", [P, TC * 8], i16, kind="ExternalInput").ap()
    t_m = nc.dram_tensor("m", [P, TC], bf16, kind="ExternalInput").ap()
    t_W1 = nc.dram_tensor("W1full", [F_IN, 80], bf16, kind="ExternalInput").ap()
    t_W2 = nc.dram_tensor("W2full", [OUT1, 66], bf16, kind="ExternalInput").ap()
    t_out = nc.dram_tensor("out", [N_PAD, NLAB], f32, kind="ExternalOutput").ap()

    KCH = F_IN // P  # 4 k-chunks

    with tile.TileContext(nc) as tc:
        with tc.tile_pool(name="dram", bufs=1, space="DRAM") as dram, \
             tc.tile_pool(name="const", bufs=1) as cpool, \
             tc.tile_pool(name="work", bufs=3) as wpool, \
             tc.tile_pool(name="gath", bufs=2) as gpool, \
             tc.tile_pool(name="msgsp", bufs=2) as mpool, \
             tc.tile_pool(name="psum", bufs=2, space="PSUM") as pp, \
             tc.tile_pool(name="psum1", bufs=2, space="PSUM") as pp1:

            ident = cpool.tile([P, P], f32)
            make_identity(nc, ident[:])

            w1_sb = cpool.tile([P, KCH, 80], bf16)
            nc.sync.dma_start(out=w1_sb[:], in_=t_W1.rearrange("(k p) w -> p k w", p=P))
            w2_sb = cpool.tile([OUT1, 66], bf16)
            nc.sync.dma_start(out=w2_sb[:], in_=t_W2[:])
            m_sb = cpool.tile([P, TC], bf16)
            nc.sync.dma_start(out=m_sb[:], in_=t_m[:])
            idx_sb = cpool.tile([P, TC * 8], i16)
            nc.sync.dma_start(out=idx_sb[:], in_=t_idx[:])

            ed1_all = cpool.tile([P, NBLK, H1], f32)
            ed2_all = cpool.tile([P, NBLK, 1], f32)

            # DRAM tables
            T1_loc = dram.tile([N_PAD, ROW], bf16)
            T2_loc = dram.tile([N_PAD, ROW], bf16)
            T1_full = dram.tile([N_TOT, ROW], bf16, addr_space="Shared")
            T2_full = dram.tile([N_TOT, ROW], bf16, addr_space="Shared")

            # ---------------- phase 1: transform layer 1 ----------------
            for b in range(NBLK):
                ps = pp.tile([P, 80], f32, tag="tf1", space="PSUM")
                for k in range(KCH):
                    xt = wpool.tile([P, P], bf16, tag="xt", bufs=4)
                    nc.sync.dma_start(out=xt[:],
                                      in_=t_xT[k * P:(k + 1) * P, b * P:(b + 1) * P])
                    nc.tensor.matmul(out=ps[:], lhsT=xt[:],
                                     rhs=w1_sb[:, k, :], start=(k == 0), stop=(k == KCH - 1))
                row = wpool.tile([P, ROW], bf16, tag="trow")
                nc.vector.memset(row[:], 0.0)
                nc.vector.tensor_copy(out=row[:, 0:72], in_=ps[:, 0:72])
                if b == NBLK - 1:
                    # pad nodes (tail partitions) must have es = -1e30
                    npad = N_PAD - N_PC  # 22
                    nc.gpsimd.affine_select(
                        out=row[:, ES_OFF:ES_OFF + H1], in_=row[:, ES_OFF:ES_OFF + H1],
                        pattern=[[0, H1]], compare_op=mybir.AluOpType.is_ge,
                        fill=NEG_BIG, base=P - npad - 1, channel_multiplier=-1)
                nc.sync.dma_start(out=T1_loc[b * P:(b + 1) * P, :], in_=row[:])
                nc.vector.tensor_copy(out=ed1_all[:, b, :], in_=ps[:, 72:80])

            # ---------------- all-gather table 1 ----------------
            nc.gpsimd.collective_compute(
                "AllGather", mybir.AluOpType.bypass,
                replica_groups=[list(range(NCORES))],
                ins=[T1_loc[:].opt()], outs=[T1_full[:].opt()])

            # pair view of the full table: [NPAIR, 256]
            T1_pair = T1_full[:].rearrange("(q t) r -> q (t r)", t=2)
            T2_pair = T2_full[:].rearrange("(q t) r -> q (t r)", t=2)

            def edge_block(b, T_pair, ed_ap, H, layer):
                """Process dst-block b for one layer. Returns (numer, den) tiles."""
                cap = caps[b]; o0 = slot_off[b]
                ni = cap * P
                g = gpool.tile([P, caps[0], 256], bf16, tag="g")
                n_g = min((ni + MAX_GATHER - 1) // MAX_GATHER, 8)
                step = ((cap + n_g - 1) // n_g)
                c0 = 0
                while c0 < cap:
                    c1 = min(c0 + step, cap)
                    nc.gpsimd.dma_gather(
                        out_ap=g[:, c0:c1, :], in_ap=T_pair,
                        idxs_ap=idx_sb[:, (o0 + c0) * 8:(o0 + c1) * 8],
                        num_idxs=(c1 - c0) * P, num_idxs_reg=(c1 - c0) * P,
                        elem_size=256, single_packet=False)
                    c0 = c1
                mm = m_sb[:, o0:o0 + cap]

                # es select + ed + lrelu + exp
                esA = g[:, 0:cap, ES_OFF:ES_OFF + H]
                esB = g[:, 0:cap, 128 + ES_OFF:128 + ES_OFF + H]
                d1 = wpool.tile([P, caps[0], H1], f32, tag="d1", name="d1t")[:, 0:cap, 0:H]
                nc.vector.tensor_tensor(out=d1, in0=esB, in1=esA, op=AL.subtract)
                t1 = wpool.tile([P, caps[0], H1], f32, tag="tt1", name="tt1t")[:, 0:cap, 0:H]
                nc.vector.tensor_tensor(out=t1, in0=d1,
                                        in1=mm.unsqueeze(-1).broadcast_to([P, cap, H]),
                                        op=AL.mult)
                s = wpool.tile([P, caps[0], H1], f32, tag="s", name="st")[:, 0:cap, 0:H]
                nc.vector.tensor_tensor(out=s, in0=esA, in1=t1, op=AL.add)
                nc.vector.tensor_tensor(out=s, in0=s,
                                        in1=ed_ap.unsqueeze(1).broadcast_to([P, cap, H]),
                                        op=AL.add)
                e = wpool.tile([P, caps[0], H1], f32, tag="e", name="et")[:, 0:cap, 0:H]
                nc.scalar.activation(out=e, in_=s, func=AF.Lrelu, alpha=LRELU_SLOPE)
                ex = wpool.tile([P, caps[0], H1], f32, tag="exx", name="exxt")[:, 0:cap, 0:H]
                nc.scalar.activation(out=ex, in_=e, func=AF.Exp)

                # exps[p, j, half, h]: exB' = ex*m, exA' = ex - exB'
                exps = wpool.tile([P, caps[0], 2, H1], bf16, tag="exps", name="expst")[:, 0:cap, :, 0:H]
                nc.vector.tensor_tensor(out=exps[:, :, 1, :], in0=ex,
                                        in1=mm.unsqueeze(-1).broadcast_to([P, cap, H]),
                                        op=AL.mult)
                nc.vector.tensor_tensor(out=exps[:, :, 0, :], in0=ex, in1=exps[:, :, 1, :],
                                        op=AL.subtract)

                # msgs[p, j, half, h, o] = Wh[p, j, half, h, o] * exps[p, j, half, h]
                OUTD = 64
                msgs = mpool.tile([P, caps[0], 2, OUTD], f32, tag="msgs", name="msgst")[:, 0:cap, :, :]
                wh = g[:, 0:cap, :].rearrange("p j (t r) -> p j t r", t=2)[:, :, :, 0:OUTD]
                if H > 1:
                    wh5 = wh.rearrange("p j t (h o) -> p j t h o", h=H)
                    ex5 = exps.unsqueeze(-1).broadcast_to([P, cap, 2, H, OUTD // H])
                    nc.vector.tensor_tensor(out=msgs.rearrange("p j t (h o) -> p j t h o", h=H),
                                            in0=wh5, in1=ex5, op=AL.mult)
                else:
                    ex4 = exps.broadcast_to([P, cap, 2, OUTD])
                    nc.vector.tensor_tensor(out=msgs, in0=wh, in1=ex4, op=AL.mult)

                numer = wpool.tile([P, OUTD], f32, tag="num")
                # reduce over (j, half) keeping (h, o): innermost = combined (j,t)
                nc.vector.tensor_reduce(
                    out=numer[:], in_=msgs.rearrange("p j t r -> p r (j t)"),
                    axis=AX.X, op=AL.add)
                den = wpool.tile([P, H1], f32, tag="den", name="dent")[:, 0:H]
                nc.vector.tensor_reduce(out=den, in_=exps.rearrange("p j t h -> p h (j t)"),
                                        axis=AX.X, op=AL.add)
                nc.vector.tensor_scalar_add(out=den, in0=den, scalar1=1e-10)
                rec = wpool.tile([P, H1], f32, tag="rec", name="rect")[:, 0:H]
                nc.vector.reciprocal(out=rec, in_=den)
                hpre = wpool.tile([P, OUTD], f32, tag="hpre")
                if H > 1:
                    nc.vector.tensor_tensor(
                        out=hpre[:].rearrange("p (h o) -> p h o", h=H),
                        in0=numer[:].rearrange("p (h o) -> p h o", h=H),
                        in1=rec.unsqueeze(-1).broadcast_to([P, H, OUTD // H]), op=AL.mult)
                else:
                    nc.vector.tensor_tensor(out=hpre[:], in0=numer[:],
                                            in1=rec.broadcast_to([P, OUTD]), op=AL.mult)
                return hpre

            # ---------------- phase 2: layer-1 edges + layer-2 transform ----------------
            for b in range(NBLK):
                hpre = edge_block(b, T1_pair, ed1_all[:, b, :], H1, 1)
                # ELU: h = relu(x) + min(exp(x),1) - 1
                ex_h = wpool.tile([P, OUT1], f32, tag="eluex")
                nc.scalar.activation(out=ex_h[:], in_=hpre[:], func=AF.Exp)
                nc.vector.tensor_scalar_min(out=ex_h[:], in0=ex_h[:], scalar1=1.0)
                r_h = wpool.tile([P, OUT1], f32, tag="elur")
                nc.vector.tensor_scalar_max(out=r_h[:], in0=hpre[:], scalar1=0.0)
                h = wpool.tile([P, OUT1], f32, tag="hfin")
                nc.vector.tensor_tensor(out=h[:], in0=r_h[:], in1=ex_h[:], op=AL.add)
                nc.vector.tensor_scalar_add(out=h[:], in0=h[:], scalar1=-1.0)
                # transpose h -> [64, 128]
                hT_ps = pp1.tile([OUT1, P], f32, tag="hT", space="PSUM")
                nc.tensor.transpose(out=hT_ps[:], in_=h[:], identity=ident[:])
                hT = wpool.tile([OUT1, P], bf16, tag="hTb")
                nc.vector.tensor_copy(out=hT[:], in_=hT_ps[:])
                # layer-2 transform
                ps2 = pp.tile([P, 66], f32, tag="tf2", space="PSUM")
                nc.tensor.matmul(out=ps2[:], lhsT=hT[:], rhs=w2_sb[:], start=True, stop=True)
                row2 = wpool.tile([P, ROW], bf16, tag="trow")
                nc.vector.memset(row2[:], 0.0)
                nc.vector.tensor_copy(out=row2[:, 0:65], in_=ps2[:, 0:65])
                if b == NBLK - 1:
                    npad = N_PAD - N_PC
                    nc.gpsimd.affine_select(
                        out=row2[:, ES_OFF:ES_OFF + 1], in_=row2[:, ES_OFF:ES_OFF + 1],
                        pattern=[[0, 1]], compare_op=mybir.AluOpType.is_ge,
                        fill=NEG_BIG, base=P - npad - 1, channel_multiplier=-1)
                nc.sync.dma_start(out=T2_loc[b * P:(b + 1) * P, :], in_=row2[:])
                nc.vector.tensor_copy(out=ed2_all[:, b, :], in_=ps2[:, 65:66])

            # ---------------- all-gather table 2 ----------------
            nc.gpsimd.collective_compute(
                "AllGather", mybir.AluOpType.bypass,
                replica_groups=[list(range(NCORES))],
                ins=[T2_loc[:].opt()], outs=[T2_full[:].opt()])

            # ---------------- phase 3: layer-2 edges + softmax ----------------
            for b in range(NBLK):
                opre = edge_block(b, T2_pair, ed2_all[:, b, :], H2, 2)
                rm = wpool.tile([P, 1], f32, tag="rm")
                nc.vector.tensor_reduce(out=rm[:], in_=opre[:], axis=AX.X,
                                        op=AL.max, negate=True)
                z = wpool.tile([P, NLAB], f32, tag="z")
                zsum = wpool.tile([P, 1], f32, tag="zsum")
                nc.scalar.activation(out=z[:], in_=opre[:], func=AF.Exp,
                                     bias=rm[:], accum_out=zsum[:])
                recs = wpool.tile([P, 1], f32, tag="recs")
                nc.vector.reciprocal(out=recs[:], in_=zsum[:])
                fin = wpool.tile([P, NLAB], f32, tag="fin")
                nc.vector.tensor_tensor(out=fin[:], in0=z[:],
                                        in1=recs[:].broadcast_to([P, NLAB]), op=AL.mult)
                nc.sync.dma_start(out=t_out[b * P:(b + 1) * P, :], in_=fin[:])

    nc.compile()
    return nc


def _install_ntff_shim():
    """antenv.axon_hooks is absent in this image; register the NTFF profile
    hook so trace=True can capture exec times. No-op if already present."""
    import types
    try:
        import antenv.axon_hooks  # noqa: F401
        return
    except ImportError:
        pass
    try:
        import antenv
        from trn_agent_boot.trn_boot import _ntff_profile_via_ctypes
        mod = types.ModuleType("antenv.axon_hooks")
        mod._hook = _ntff_profile_via_ctypes('/opt/axon/libaxon_pjrt.so')
        mod.set_axon_ntff_profile_hook = lambda h: setattr(mod, "_hook", h)
        mod.get_axon_ntff_profile_hook = lambda: mod._hook
        sys.modules["antenv.axon_hooks"] = mod
        antenv.axon_hooks = mod
    except Exception:
        pass


def kernel(inputs, W1, a1_src, a1_dst, W2, a2_src, a2_dst, src, dst):
    from concourse import bass_utils
    if int(os.environ.get("GAT_TRACE", "0")):
        _install_ntff_shim()
    meta, per_core, shared = host_prep(inputs, W1, a1_src, a1_dst, W2,
                                       a2_src, a2_dst, src, dst)
    nc = build_kernel(meta)
    in_maps = []
    for c in range(NCORES):
        in_maps.append(dict(
            xT=per_core['xT'][c], idx=per_core['idx'][c], m=per_core['m'][c],
            W1full=shared['W1full'], W2full=shared['W2full']))
    res = bass_utils.run_bass_kernel_spmd(
        nc, in_maps, core_ids=list(range(NCORES)),
        trace=bool(int(os.environ.get("GAT_TRACE", "0"))),
        trace_cores=list(range(NCORES)) if int(os.environ.get("GAT_TRACE", "0")) else None)
    kernel.last_exec_time_ns = res.exec_time_ns
    out = np.zeros((N, NLAB), dtype=np.float32)
    for c in range(NCORES):
        o = res.results[c]["out"]
        perm = meta['perms'][c]
        real = perm >= 0
        out[perm[real]] = o[real]
    return out


def mirror(inputs, W1, a1_src, a1_dst, W2, a2_src, a2_dst, src, dst):
    """Numpy mirror of the kernel's exact dataflow (incl. bf16 rounding of
    tables) for layout validation without hardware."""
    meta, per_core, shared = host_prep(inputs, W1, a1_src, a1_dst, W2,
                                       a2_src, a2_dst, src, dst)
    caps = meta['caps']; slot_off = meta['slot_off']; TC = meta['tot_cols']
    W1full = shared['W1full'].astype(np.float32)
    W2full = shared['W2full'].astype(np.float32)
    out = np.zeros((N, NLAB), dtype=np.float32)

    # build tables per core, then allgather
    T1 = np.zeros((N_TOT, ROW), dtype=np.float32)
    ed1 = np.zeros((NCORES, N_PAD, H1), dtype=np.float32)
    for c in range(NCORES):
        xT = per_core['xT'][c].astype(np.float32)
        t = xT.T @ W1full                       # [N_PAD, 80]
        rows = np.zeros((N_PAD, ROW), np.float32)
        rows[:, 0:72] = t[:, 0:72]
        rows[N_PC:, ES_OFF:ES_OFF + H1] = NEG_BIG
        T1[c * N_PAD:(c + 1) * N_PAD] = _bf16(rows).astype(np.float32)
        ed1[c] = t[:, 72:80]

    def edge_phase(c, T, ed, H):
        Tp = T.reshape(NPAIR, 256)
        idx = per_core['idx'][c]
        m = per_core['m'][c].astype(np.float32)
        res = np.zeros((N_PAD, OUT1), np.float32)
        for b in range(NBLK):
            cap = caps[b]; o0 = slot_off[b]
            # unwrap idx: stored wrapped per block
            wr = idx[:16, o0 * 8:(o0 + cap) * 8]
            flat = wr.T.reshape(-1)             # undo .reshape(-1,16).T
            g = Tp[flat.astype(np.int64)].reshape(cap, P, 256).transpose(1, 0, 2)
            mm = m[:, o0:o0 + cap]
            esA = g[:, :, ES_OFF:ES_OFF + H]
            esB = g[:, :, 128 + ES_OFF:128 + ES_OFF + H]
            es = esA + mm[:, :, None] * (esB - esA)
            s = es + ed[b * P:(b + 1) * P].reshape(P, 1, H)
            e = np.where(s > 0, s, LRELU_SLOPE * s)
            ex = np.exp(e)
            exB = ex * mm[:, :, None]; exA = ex - exB
            whA = g[:, :, 0:64]; whB = g[:, :, 128:192]
            if H > 1:
                o = OUT1 // H
                msA = whA.reshape(P, cap, H, o) * exA[:, :, :, None]
                msB = whB.reshape(P, cap, H, o) * exB[:, :, :, None]
                numer = (msA + msB).sum(axis=1).reshape(P, OUT1)
                den = (exA + exB).sum(axis=1)
                hpre = (numer.reshape(P, H, o) / (den[:, :, None] + 1e-10)).reshape(P, OUT1)
            else:
                msA = whA * exA; msB = whB * exB
                numer = (msA + msB).sum(axis=1)
                den = (exA + exB).sum(axis=1)
                hpre = numer / (den + 1e-10)
            res[b * P:(b + 1) * P] = hpre
        return res

    T2 = np.zeros((N_TOT, ROW), dtype=np.float32)
    ed2 = np.zeros((NCORES, N_PAD, 1), dtype=np.float32)
    h_all = {}
    for c in range(NCORES):
        hpre = edge_phase(c, T1, ed1[c], H1)
        h = np.maximum(hpre, 0) + np.minimum(np.exp(hpre), 1.0) - 1.0
        h_all[c] = h
        t2 = _bf16(h).astype(np.float32) @ W2full
        rows = np.zeros((N_PAD, ROW), np.float32)
        rows[:, 0:65] = t2[:, 0:65]
        rows[N_PC:, ES_OFF:ES_OFF + 1] = NEG_BIG
        T2[c * N_PAD:(c + 1) * N_PAD] = _bf16(rows).astype(np.float32)
        ed2[c] = t2[:, 65:66]

    for c in range(NCORES):
        opre = edge_phase(c, T2, ed2[c], H2)[:, 0:NLAB]
        z = np.exp(opre - opre.max(axis=1, keepdims=True))
        fin = z / z.sum(axis=1, keepdims=True)
        perm = meta['perms'][c]; real = perm >= 0
        out[perm[real]] = fin[real]
    return out


# revision 8
# speedup vs baseline: 1.1076x; 1.1032x over previous
"""GAT 2-layer kernel for 8 Trainium2 NeuronCores (Bass/Tile).

Strategy (1D partition by dst):
- Each core owns N/8 dst nodes (padded to blocks of 128). Host permutes each
  core's nodes so in-degrees are sorted descending -> per-block edge-slot
  capacity cap_b (= cross-core max block degree) stays tight.
- Transform phase: per node-block matmul x @ [W | W@a_src | W@a_dst] gives
  [Wh | es | ed] in one pass. [Wh|es] rows (bf16, 128 elems = 256B) form the
  gather table; ed stays on-chip (per-partition, dst-local).
- AllGather the per-core tables -> every core holds the full [N, 128] bf16
  table in DRAM.
- Edge phase: per dst-block one dma_gather with PAIR rows (elem 256 bf16 =
  512B covering nodes 2k/2k+1, idx = src>>1 fits int16); edge slot (p, j) =
  j-th in-edge of the block's p-th dst. Half-select is folded into the
  attention scaling (exA = ex*(1-m), exB = ex*m).
- Attention: e = lrelu(es_sel + ed), ex = exp(e) (max-subtraction is skipped:
  |e| is bounded by a few units so exp cannot overflow; softmax is shift
  invariant). numer = sum_j ex*Wh via one multiply + one strided reduce;
  den = sum_j ex. Pad slots point at a pad pair-row whose es = -1e30 -> ex=0.
- Layer 2 reuses the same edge slots/indices; final row softmax on chip.
"""

import sys, os
sys.path.insert(0, '/opt/trn_rl_repo')

import numpy as np

# ---- problem constants (from the reference; hardcoded, not read from disk) ----
N = 50000
E = 800000
F_IN = 512
H1 = 8
F_HID = 8
OUT1 = H1 * F_HID          # 64
H2 = 1
NLAB = 64
LRELU_SLOPE = 0.2
NCORES = 8
P = 128

N_PC = N // NCORES         # 6250
NBLK = (N_PC + P - 1) // P # 49
N_PAD = NBLK * P           # 6272
N_TOT = N_PAD * NCORES     # 50176
NPAIR = N_TOT // 2         # 25088
ROW = 128                  # table row elems (bf16): [Wh 64 | es 8 | pad]
ES_OFF = 64
NEG_BIG = -1.0e30
MAX_GATHER = 8192          # HW limit for one dma_gather


def _bf16(x):
    import jax.numpy as jnp
    return np.asarray(jnp.asarray(np.asarray(x), dtype=jnp.bfloat16))


def host_prep(inputs, W1, a1_src, a1_dst, W2, a2_src, a2_dst, src, dst):
    """Pure-numpy preparation of all per-core tensors + layout metadata."""
    inputs = np.asarray(inputs); src = np.asarray(src); dst = np.asarray(dst)
    W1 = np.asarray(W1); W2 = np.asarray(W2)
    a1_src = np.asarray(a1_src); a1_dst = np.asarray(a1_dst)
    a2_src = np.asarray(a2_src); a2_dst = np.asarray(a2_dst)

    core_of = dst // N_PC                      # owner core per edge
    # per-core node permutation: sort own nodes by in-degree desc (pads last)
    perms = []          # perms[c][i] = original node id at permuted position i
    inv_pos = np.zeros(N, dtype=np.int64)      # node -> position within its core
    degs = np.bincount(dst, minlength=N)
    for c in range(NCORES):
        own = np.arange(c * N_PC, (c + 1) * N_PC)
        order = own[np.argsort(-degs[own], kind='stable')]
        perm = np.concatenate([order, np.full(N_PAD - N_PC, -1, dtype=np.int64)])
        perms.append(perm)
        inv_pos[order] = np.arange(N_PC)

    # global table row of node n (pad rows belong to tail positions)
    def table_row(n):
        return (n // N_PC) * N_PAD + inv_pos[n]

    # per-core per-block caps (cross-core max) ------------------------------
    deg_mat = np.zeros((NCORES, N_PAD), dtype=np.int64)
    for c in range(NCORES):
        real = perms[c] >= 0
        deg_mat[c, real] = degs[perms[c][real]]
    caps = deg_mat.reshape(NCORES, NBLK, P).max(axis=2).max(axis=0)  # [NBLK]
    caps = np.maximum(caps, 1)
    slot_off = np.concatenate([[0], np.cumsum(caps)])                # col offsets
    tot_cols = int(slot_off[-1])

    # pad pair-row: last two table rows are core-7 pad nodes (deg 0, x rows 0)
    PAD_PAIR = (N_TOT - 2) // 2

    # per-core edge slot assignment -----------------------------------------
    idx_all = np.full((NCORES, P, tot_cols), PAD_PAIR, dtype=np.int64)
    m_all = np.zeros((NCORES, P, tot_cols), dtype=np.float32)
    tr_src = table_row(src)
    pos_in_core = inv_pos[dst]                 # permuted position of dst
    for c in range(NCORES):
        sel = core_of == c
        s_rows = tr_src[sel]
        d_pos = pos_in_core[sel]
        order = np.argsort(d_pos, kind='stable')
        s_rows = s_rows[order]; d_pos = d_pos[order]
        # j-th edge of each dst
        jj = np.arange(len(d_pos)) - np.searchsorted(d_pos, d_pos, side='left')
        b = d_pos // P; pp = d_pos % P
        cols = slot_off[b] + jj
        assert (jj < caps[b]).all()
        idx_all[c, pp, cols] = s_rows // 2
        m_all[c, pp, cols] = (s_rows % 2).astype(np.float32)

    # wrapped int16 index layout per block: flat order i=(col*128+p),
    # reshape(-1,16).T, tiled 8x over partitions
    idx_wrapped = np.zeros((NCORES, P, tot_cols * 8), dtype=np.int16)
    for c in range(NCORES):
        for b in range(NBLK):
            o0, o1 = slot_off[b], slot_off[b + 1]
            flat = idx_all[c][:, o0:o1].T.reshape(-1)      # (col, p) order
            wr = np.tile(flat.reshape(-1, 16).T, (8, 1)).astype(np.int16)
            idx_wrapped[c][:, o0 * 8:o1 * 8] = wr

    # weights ---------------------------------------------------------------
    W1cat = W1.transpose(1, 0, 2).reshape(F_IN, OUT1)      # [512, 64]
    ws1 = np.einsum('hfo,ho->fh', W1, a1_src)              # [512, 8]
    wd1 = np.einsum('hfo,ho->fh', W1, a1_dst)              # [512, 8]
    W1full = np.concatenate([W1cat, ws1, wd1], axis=1)     # [512, 80]
    W2cat = W2.transpose(1, 0, 2).reshape(OUT1, NLAB)      # [64, 64]
    ws2 = np.einsum('hfo,ho->fh', W2, a2_src)              # [64, 1]
    wd2 = np.einsum('hfo,ho->fh', W2, a2_dst)              # [64, 1]
    W2full = np.concatenate([W2cat, ws2, wd2], axis=1)     # [64, 66]

    # per-core transposed inputs (permuted, padded) -------------------------
    xT = np.zeros((NCORES, F_IN, N_PAD), dtype=np.float32)
    for c in range(NCORES):
        real = perms[c] >= 0
        xT[c][:, real] = inputs[perms[c][real]].T

    meta = dict(caps=caps, slot_off=slot_off, tot_cols=tot_cols, perms=perms)
    per_core = dict(
        xT=[_bf16(xT[c]) for c in range(NCORES)],
        idx=[idx_wrapped[c] for c in range(NCORES)],
        m=[_bf16(m_all[c]) for c in range(NCORES)],
    )
    shared = dict(W1full=_bf16(W1full), W2full=_bf16(W2full))
    return meta, per_core, shared


def build_kernel(meta):
    import concourse.bass as bass
    import concourse.bacc as bacc
    import concourse.tile as tile
    from concourse import mybir
    from concourse.masks import make_identity

    bf16 = mybir.dt.bfloat16; f32 = mybir.dt.float32; i16 = mybir.dt.int16
    AL = mybir.AluOpType; AF = mybir.ActivationFunctionType; AX = mybir.AxisListType

    caps = [int(x) for x in meta['caps']]
    slot_off = [int(x) for x in meta['slot_off']]
    TC = int(meta['tot_cols'])

    nc = bacc.Bacc("TRN2", target_bir_lowering=False, debug=False,
                   enable_asserts=True, num_devices=NCORES)

    t_xT = nc.dram_tensor("xT", [F_IN, N_PAD], bf16, kind="ExternalInput").ap()
    t_idx = nc.dram_tensor("idx", [P, TC * 8], i16, kind="ExternalInput").ap()
    t_m = nc.dram_tensor("m", [P, TC], bf16, kind="ExternalInput").ap()
    t_W1 = nc.dram_tensor("W1full", [F_IN, 80], bf16, kind="ExternalInput").ap()
    t_W2 = nc.dram_tensor("W2full", [OUT1, 66], bf16, kind="ExternalInput").ap()
    t_out = nc.dram_tensor("out", [N_PAD, NLAB], f32, kind="ExternalOutput").ap()

    KCH = F_IN // P  # 4 k-chunks

    with tile.TileContext(nc) as tc:
        with tc.tile_pool(name="dram", bufs=1, space="DRAM") as dram, \
             tc.tile_pool(name="const", bufs=1) as cpool, \
             tc.tile_pool(name="work", bufs=3) as wpool, \
             tc.tile_pool(name="gath", bufs=3) as gpool, \
             tc.tile_pool(name="msgsp", bufs=2) as mpool, \
             tc.tile_pool(name="psum", bufs=2, space="PSUM") as pp, \
             tc.tile_pool(name="psum1", bufs=2, space="PSUM") as pp1:

            ident = cpool.tile([P, P], f32)
            make_identity(nc, ident[:])

            w1_sb = cpool.tile([P, KCH, 80], bf16)
            nc.sync.dma_start(out=w1_sb[:], in_=t_W1.rearrange("(k p) w -> p k w", p=P))
            w2_sb = cpool.tile([OUT1, 66], bf16)
            nc.sync.dma_start(out=w2_sb[:], in_=t_W2[:])
            m_sb = cpool.tile([P, TC], bf16)
            nc.sync.dma_start(out=m_sb[:], in_=t_m[:])
            idx_sb = cpool.tile([P, TC * 8], i16)
            nc.sync.dma_start(out=idx_sb[:], in_=t_idx[:])

            ed1_all = cpool.tile([P, NBLK, H1], f32)
            ed2_all = cpool.tile([P, NBLK, 1], f32)

            # DRAM tables
            T1_loc = dram.tile([N_PAD, ROW], bf16)
            T2_loc = dram.tile([N_PAD, ROW], bf16)
            T1_full = dram.tile([N_TOT, ROW], bf16, addr_space="Shared")
            T2_full = dram.tile([N_TOT, ROW], bf16, addr_space="Shared")

            # ---------------- phase 1: transform layer 1 ----------------
            for b in range(NBLK):
                ps = pp.tile([P, 80], f32, tag="tf1", space="PSUM")
                for k in range(KCH):
                    xt = wpool.tile([P, P], bf16, tag="xt", bufs=4)
                    nc.sync.dma_start(out=xt[:],
                                      in_=t_xT[k * P:(k + 1) * P, b * P:(b + 1) * P])
                    nc.tensor.matmul(out=ps[:], lhsT=xt[:],
                                     rhs=w1_sb[:, k, :], start=(k == 0), stop=(k == KCH - 1))
                row = wpool.tile([P, ROW], bf16, tag="trow")
                nc.vector.memset(row[:], 0.0)
                nc.vector.tensor_copy(out=row[:, 0:72], in_=ps[:, 0:72])
                if b == NBLK - 1:
                    # pad nodes (tail partitions) must have es = -1e30
                    npad = N_PAD - N_PC  # 22
                    nc.gpsimd.affine_select(
                        out=row[:, ES_OFF:ES_OFF + H1], in_=row[:, ES_OFF:ES_OFF + H1],
                        pattern=[[0, H1]], compare_op=mybir.AluOpType.is_ge,
                        fill=NEG_BIG, base=P - npad - 1, channel_multiplier=-1)
                nc.sync.dma_start(out=T1_loc[b * P:(b + 1) * P, :], in_=row[:])
                nc.vector.tensor_copy(out=ed1_all[:, b, :], in_=ps[:, 72:80])

            # ---------------- all-gather table 1 ----------------
            nc.gpsimd.collective_compute(
                "AllGather", mybir.AluOpType.bypass,
                replica_groups=[list(range(NCORES))],
                ins=[T1_loc[:].opt()], outs=[T1_full[:].opt()])

            # pair view of the full table: [NPAIR, 256]
            T1_pair = T1_full[:].rearrange("(q t) r -> q (t r)", t=2)
            T2_pair = T2_full[:].rearrange("(q t) r -> q (t r)", t=2)

            def edge_block(b, T_pair, ed_ap, H, layer):
                """Process dst-block b for one layer. Returns (numer, den) tiles."""
                cap = caps[b]; o0 = slot_off[b]
                ni = cap * P
                g = gpool.tile([P, caps[0], 256], bf16, tag="g")
                n_g = min((ni + MAX_GATHER - 1) // MAX_GATHER, 8)
                step = ((cap + n_g - 1) // n_g)
                c0 = 0
                while c0 < cap:
                    c1 = min(c0 + step, cap)
                    nc.gpsimd.dma_gather(
                        out_ap=g[:, c0:c1, :], in_ap=T_pair,
                        idxs_ap=idx_sb[:, (o0 + c0) * 8:(o0 + c1) * 8],
                        num_idxs=(c1 - c0) * P, num_idxs_reg=(c1 - c0) * P,
                        elem_size=256, single_packet=False)
                    c0 = c1
                mm = m_sb[:, o0:o0 + cap]

                # es select + ed + lrelu + exp
                esA = g[:, 0:cap, ES_OFF:ES_OFF + H]
                esB = g[:, 0:cap, 128 + ES_OFF:128 + ES_OFF + H]
                d1 = wpool.tile([P, caps[0], H1], f32, tag="d1", name="d1t")[:, 0:cap, 0:H]
                nc.vector.tensor_tensor(out=d1, in0=esB, in1=esA, op=AL.subtract)
                t1 = wpool.tile([P, caps[0], H1], f32, tag="tt1", name="tt1t")[:, 0:cap, 0:H]
                nc.vector.tensor_tensor(out=t1, in0=d1,
                                        in1=mm.unsqueeze(-1).broadcast_to([P, cap, H]),
                                        op=AL.mult)
                s = wpool.tile([P, caps[0], H1], f32, tag="s", name="st")[:, 0:cap, 0:H]
                nc.vector.tensor_tensor(out=s, in0=esA, in1=t1, op=AL.add)
                nc.vector.tensor_tensor(out=s, in0=s,
                                        in1=ed_ap.unsqueeze(1).broadcast_to([P, cap, H]),
                                        op=AL.add)
                e = wpool.tile([P, caps[0], H1], f32, tag="e", name="et")[:, 0:cap, 0:H]
                mx = wpool.tile([P, caps[0], H1], f32, tag="mx", name="mxt")[:, 0:cap, 0:H]
                nc.vector.tensor_scalar_max(out=mx, in0=s, scalar1=0.0)
                nc.vector.tensor_scalar_min(out=e, in0=s, scalar1=0.0)
                nc.vector.scalar_tensor_tensor(out=e, in0=e, scalar=LRELU_SLOPE,
                                               in1=mx, op0=AL.mult, op1=AL.add)
                ex = wpool.tile([P, caps[0], H1], f32, tag="exx", name="exxt")[:, 0:cap, 0:H]
                nc.scalar.activation(out=ex, in_=e, func=AF.Exp)

                # exps[p, j, half, h]: exB' = ex*m, exA' = ex - exB'
                exps = wpool.tile([P, caps[0], 2, H1], bf16, tag="exps", name="expst")[:, 0:cap, :, 0:H]
                nc.vector.tensor_tensor(out=exps[:, :, 1, :], in0=ex,
                                        in1=mm.unsqueeze(-1).broadcast_to([P, cap, H]),
                                        op=AL.mult)
                nc.vector.tensor_tensor(out=exps[:, :, 0, :], in0=ex, in1=exps[:, :, 1, :],
                                        op=AL.subtract)

                # msgs[p, j, half, h, o] = Wh[p, j, half, h, o] * exps[p, j, half, h]
                OUTD = 64
                msgs = mpool.tile([P, caps[0], 2, OUTD], bf16, tag="msgs", name="msgst")[:, 0:cap, :, :]
                wh = g[:, 0:cap, :].rearrange("p j (t r) -> p j t r", t=2)[:, :, :, 0:OUTD]
                if H > 1:
                    wh5 = wh.rearrange("p j t (h o) -> p j t h o", h=H)
                    ex5 = exps.unsqueeze(-1).broadcast_to([P, cap, 2, H, OUTD // H])
                    nc.vector.tensor_tensor(out=msgs.rearrange("p j t (h o) -> p j t h o", h=H),
                                            in0=wh5, in1=ex5, op=AL.mult)
                else:
                    ex4 = exps.broadcast_to([P, cap, 2, OUTD])
                    nc.vector.tensor_tensor(out=msgs, in0=wh, in1=ex4, op=AL.mult)

                # tree-reduce over (j, half) with contiguous chunk adds
                tre = mpool.tile([P, caps[0], OUTD], f32, tag="tree", name="treet")
                nc.vector.tensor_tensor(out=tre[:, 0:cap, :], in0=msgs[:, :, 0, :],
                                        in1=msgs[:, :, 1, :], op=AL.add)
                cur = cap
                while cur > 1:
                    mhalf = cur // 2
                    nc.vector.tensor_tensor(out=tre[:, 0:mhalf, :], in0=tre[:, 0:mhalf, :],
                                            in1=tre[:, cur - mhalf:cur, :], op=AL.add)
                    cur -= mhalf
                numer = tre[:, 0, :]
                den = wpool.tile([P, H1], f32, tag="den", name="dent")[:, 0:H]
                nc.vector.tensor_reduce(out=den, in_=exps.rearrange("p j t h -> p h (j t)"),
                                        axis=AX.X, op=AL.add)
                nc.vector.tensor_scalar_add(out=den, in0=den, scalar1=1e-10)
                rec = wpool.tile([P, H1], f32, tag="rec", name="rect")[:, 0:H]
                nc.vector.reciprocal(out=rec, in_=den)
                hpre = wpool.tile([P, OUTD], f32, tag="hpre")
                if H > 1:
                    nc.vector.tensor_tensor(
                        out=hpre[:].rearrange("p (h o) -> p h o", h=H),
                        in0=numer.rearrange("p (h o) -> p h o", h=H),
                        in1=rec.unsqueeze(-1).broadcast_to([P, H, OUTD // H]), op=AL.mult)
                else:
                    nc.vector.tensor_tensor(out=hpre[:], in0=numer,
                                            in1=rec.broadcast_to([P, OUTD]), op=AL.mult)
                return hpre

            # ---------------- phase 2: layer-1 edges + layer-2 transform ----------------
            for b in range(NBLK):
                hpre = edge_block(b, T1_pair, ed1_all[:, b, :], H1, 1)
                # ELU: h = relu(x) + min(exp(x),1) - 1
                ex_h = wpool.tile([P, OUT1], f32, tag="eluex")
                nc.scalar.activation(out=ex_h[:], in_=hpre[:], func=AF.Exp)
                nc.vector.tensor_scalar_min(out=ex_h[:], in0=ex_h[:], scalar1=1.0)
                r_h = wpool.tile([P, OUT1], f32, tag="elur")
                nc.vector.tensor_scalar_max(out=r_h[:], in0=hpre[:], scalar1=0.0)
                h = wpool.tile([P, OUT1], f32, tag="hfin")
                nc.vector.tensor_tensor(out=h[:], in0=r_h[:], in1=ex_h[:], op=AL.add)
                nc.vector.tensor_scalar_add(out=h[:], in0=h[:], scalar1=-1.0)
                # transpose h -> [64, 128]
                hT_ps = pp1.tile([OUT1, P], f32, tag="hT", space="PSUM")
                nc.tensor.transpose(out=hT_ps[:], in_=h[:], identity=ident[:])
                hT = wpool.tile([OUT1, P], bf16, tag="hTb")
                nc.vector.tensor_copy(out=hT[:], in_=hT_ps[:])
                # layer-2 transform
                ps2 = pp.tile([P, 66], f32, tag="tf2", space="PSUM")
                nc.tensor.matmul(out=ps2[:], lhsT=hT[:], rhs=w2_sb[:], start=True, stop=True)
                row2 = wpool.tile([P, ROW], bf16, tag="trow")
                nc.vector.memset(row2[:], 0.0)
                nc.vector.tensor_copy(out=row2[:, 0:65], in_=ps2[:, 0:65])
                if b == NBLK - 1:
                    npad = N_PAD - N_PC
                    nc.gpsimd.affine_select(
                        out=row2[:, ES_OFF:ES_OFF + 1], in_=row2[:, ES_OFF:ES_OFF + 1],
                        pattern=[[0, 1]], compare_op=mybir.AluOpType.is_ge,
                        fill=NEG_BIG, base=P - npad - 1, channel_multiplier=-1)
                nc.sync.dma_start(out=T2_loc[b * P:(b + 1) * P, :], in_=row2[:])
                nc.vector.tensor_copy(out=ed2_all[:, b, :], in_=ps2[:, 65:66])

            # ---------------- all-gather table 2 ----------------
            nc.gpsimd.collective_compute(
                "AllGather", mybir.AluOpType.bypass,
                replica_groups=[list(range(NCORES))],
                ins=[T2_loc[:].opt()], outs=[T2_full[:].opt()])

            # ---------------- phase 3: layer-2 edges + softmax ----------------
            for b in range(NBLK):
                opre = edge_block(b, T2_pair, ed2_all[:, b, :], H2, 2)
                rm = wpool.tile([P, 1], f32, tag="rm")
                nc.vector.tensor_reduce(out=rm[:], in_=opre[:], axis=AX.X,
                                        op=AL.max, negate=True)
                z = wpool.tile([P, NLAB], f32, tag="z")
                zsum = wpool.tile([P, 1], f32, tag="zsum")
                nc.scalar.activation(out=z[:], in_=opre[:], func=AF.Exp,
                                     bias=rm[:], accum_out=zsum[:])
                recs = wpool.tile([P, 1], f32, tag="recs")
                nc.vector.reciprocal(out=recs[:], in_=zsum[:])
                fin = wpool.tile([P, NLAB], f32, tag="fin")
                nc.vector.tensor_tensor(out=fin[:], in0=z[:],
                                        in1=recs[:].broadcast_to([P, NLAB]), op=AL.mult)
                nc.sync.dma_start(out=t_out[b * P:(b + 1) * P, :], in_=fin[:])

    nc.compile()
    return nc


def _install_ntff_shim():
    """antenv.axon_hooks is absent in this image; register the NTFF profile
    hook so trace=True can capture exec times. No-op if already present."""
    import types
    try:
        import antenv.axon_hooks  # noqa: F401
        return
    except ImportError:
        pass
    try:
        import antenv
        from trn_agent_boot.trn_boot import _ntff_profile_via_ctypes
        mod = types.ModuleType("antenv.axon_hooks")
        mod._hook = _ntff_profile_via_ctypes('/opt/axon/libaxon_pjrt.so')
        mod.set_axon_ntff_profile_hook = lambda h: setattr(mod, "_hook", h)
        mod.get_axon_ntff_profile_hook = lambda: mod._hook
        sys.modules["antenv.axon_hooks"] = mod
        antenv.axon_hooks = mod
    except Exception:
        pass


def kernel(inputs, W1, a1_src, a1_dst, W2, a2_src, a2_dst, src, dst):
    from concourse import bass_utils
    if int(os.environ.get("GAT_TRACE", "0")):
        _install_ntff_shim()
    meta, per_core, shared = host_prep(inputs, W1, a1_src, a1_dst, W2,
                                       a2_src, a2_dst, src, dst)
    nc = build_kernel(meta)
    in_maps = []
    for c in range(NCORES):
        in_maps.append(dict(
            xT=per_core['xT'][c], idx=per_core['idx'][c], m=per_core['m'][c],
            W1full=shared['W1full'], W2full=shared['W2full']))
    res = bass_utils.run_bass_kernel_spmd(
        nc, in_maps, core_ids=list(range(NCORES)),
        trace=bool(int(os.environ.get("GAT_TRACE", "0"))),
        trace_cores=list(range(NCORES)) if int(os.environ.get("GAT_TRACE", "0")) else None)
    kernel.last_exec_time_ns = res.exec_time_ns
    out = np.zeros((N, NLAB), dtype=np.float32)
    for c in range(NCORES):
        o = res.results[c]["out"]
        perm = meta['perms'][c]
        real = perm >= 0
        out[perm[real]] = o[real]
    return out


def mirror(inputs, W1, a1_src, a1_dst, W2, a2_src, a2_dst, src, dst):
    """Numpy mirror of the kernel's exact dataflow (incl. bf16 rounding of
    tables) for layout validation without hardware."""
    meta, per_core, shared = host_prep(inputs, W1, a1_src, a1_dst, W2,
                                       a2_src, a2_dst, src, dst)
    caps = meta['caps']; slot_off = meta['slot_off']; TC = meta['tot_cols']
    W1full = shared['W1full'].astype(np.float32)
    W2full = shared['W2full'].astype(np.float32)
    out = np.zeros((N, NLAB), dtype=np.float32)

    # build tables per core, then allgather
    T1 = np.zeros((N_TOT, ROW), dtype=np.float32)
    ed1 = np.zeros((NCORES, N_PAD, H1), dtype=np.float32)
    for c in range(NCORES):
        xT = per_core['xT'][c].astype(np.float32)
        t = xT.T @ W1full                       # [N_PAD, 80]
        rows = np.zeros((N_PAD, ROW), np.float32)
        rows[:, 0:72] = t[:, 0:72]
        rows[N_PC:, ES_OFF:ES_OFF + H1] = NEG_BIG
        T1[c * N_PAD:(c + 1) * N_PAD] = _bf16(rows).astype(np.float32)
        ed1[c] = t[:, 72:80]

    def edge_phase(c, T, ed, H):
        Tp = T.reshape(NPAIR, 256)
        idx = per_core['idx'][c]
        m = per_core['m'][c].astype(np.float32)
        res = np.zeros((N_PAD, OUT1), np.float32)
        for b in range(NBLK):
            cap = caps[b]; o0 = slot_off[b]
            # unwrap idx: stored wrapped per block
            wr = idx[:16, o0 * 8:(o0 + cap) * 8]
            flat = wr.T.reshape(-1)             # undo .reshape(-1,16).T
            g = Tp[flat.astype(np.int64)].reshape(cap, P, 256).transpose(1, 0, 2)
            mm = m[:, o0:o0 + cap]
            esA = g[:, :, ES_OFF:ES_OFF + H]
            esB = g[:, :, 128 + ES_OFF:128 + ES_OFF + H]
            es = esA + mm[:, :, None] * (esB - esA)
            s = es + ed[b * P:(b + 1) * P].reshape(P, 1, H)
            e = np.where(s > 0, s, LRELU_SLOPE * s)
            ex = np.exp(e)
            exB = ex * mm[:, :, None]; exA = ex - exB
            whA = g[:, :, 0:64]; whB = g[:, :, 128:192]
            if H > 1:
                o = OUT1 // H
                msA = whA.reshape(P, cap, H, o) * exA[:, :, :, None]
                msB = whB.reshape(P, cap, H, o) * exB[:, :, :, None]
                numer = (msA + msB).sum(axis=1).reshape(P, OUT1)
                den = (exA + exB).sum(axis=1)
                hpre = (numer.reshape(P, H, o) / (den[:, :, None] + 1e-10)).reshape(P, OUT1)
            else:
                msA = whA * exA; msB = whB * exB
                numer = (msA + msB).sum(axis=1)
                den = (exA + exB).sum(axis=1)
                hpre = numer / (den + 1e-10)
            res[b * P:(b + 1) * P] = hpre
        return res

    T2 = np.zeros((N_TOT, ROW), dtype=np.float32)
    ed2 = np.zeros((NCORES, N_PAD, 1), dtype=np.float32)
    h_all = {}
    for c in range(NCORES):
        hpre = edge_phase(c, T1, ed1[c], H1)
        h = np.maximum(hpre, 0) + np.minimum(np.exp(hpre), 1.0) - 1.0
        h_all[c] = h
        t2 = _bf16(h).astype(np.float32) @ W2full
        rows = np.zeros((N_PAD, ROW), np.float32)
        rows[:, 0:65] = t2[:, 0:65]
        rows[N_PC:, ES_OFF:ES_OFF + 1] = NEG_BIG
        T2[c * N_PAD:(c + 1) * N_PAD] = _bf16(rows).astype(np.float32)
        ed2[c] = t2[:, 65:66]

    for c in range(NCORES):
        opre = edge_phase(c, T2, ed2[c], H2)[:, 0:NLAB]
        z = np.exp(opre - opre.max(axis=1, keepdims=True))
        fin = z / z.sum(axis=1, keepdims=True)
        perm = meta['perms'][c]; real = perm >= 0
        out[perm[real]] = fin[real]
    return out


# revision 9
# speedup vs baseline: 1.2083x; 1.0909x over previous
"""GAT 2-layer kernel for 8 Trainium2 NeuronCores (Bass/Tile).

Strategy (1D partition by dst):
- Each core owns N/8 dst nodes (padded to blocks of 128). Host permutes each
  core's nodes so in-degrees are sorted descending -> per-block edge-slot
  capacity cap_b (= cross-core max block degree) stays tight.
- Transform phase: per node-block matmul x @ [W | W@a_src | W@a_dst] gives
  [Wh | es | ed] in one pass. [Wh|es] rows (bf16, 128 elems = 256B) form the
  gather table; ed stays on-chip (per-partition, dst-local).
- AllGather the per-core tables -> every core holds the full [N, 128] bf16
  table in DRAM.
- Edge phase: per dst-block one dma_gather with PAIR rows (elem 256 bf16 =
  512B covering nodes 2k/2k+1, idx = src>>1 fits int16); edge slot (p, j) =
  j-th in-edge of the block's p-th dst. Half-select is folded into the
  attention scaling (exA = ex*(1-m), exB = ex*m).
- Attention: e = lrelu(es_sel + ed), ex = exp(e) (max-subtraction is skipped:
  |e| is bounded by a few units so exp cannot overflow; softmax is shift
  invariant). numer = sum_j ex*Wh via one multiply + one strided reduce;
  den = sum_j ex. Pad slots point at a pad pair-row whose es = -1e30 -> ex=0.
- Layer 2 reuses the same edge slots/indices; final row softmax on chip.
"""

import sys, os
sys.path.insert(0, '/opt/trn_rl_repo')

import numpy as np

# ---- problem constants (from the reference; hardcoded, not read from disk) ----
N = 50000
E = 800000
F_IN = 512
H1 = 8
F_HID = 8
OUT1 = H1 * F_HID          # 64
H2 = 1
NLAB = 64
LRELU_SLOPE = 0.2
NCORES = 8
P = 128

N_PC = N // NCORES         # 6250
NBLK = (N_PC + P - 1) // P # 49
N_PAD = NBLK * P           # 6272
N_TOT = N_PAD * NCORES     # 50176
NPAIR = N_TOT // 2         # 25088
ROW = 128                  # table row elems (bf16): [Wh 64 | es 8 | pad]
ES_OFF = 64
NEG_BIG = -1.0e30
MAX_GATHER = 8192          # HW limit for one dma_gather


def _bf16(x):
    import jax.numpy as jnp
    return np.asarray(jnp.asarray(np.asarray(x), dtype=jnp.bfloat16))


def host_prep(inputs, W1, a1_src, a1_dst, W2, a2_src, a2_dst, src, dst):
    """Pure-numpy preparation of all per-core tensors + layout metadata."""
    inputs = np.asarray(inputs); src = np.asarray(src); dst = np.asarray(dst)
    W1 = np.asarray(W1); W2 = np.asarray(W2)
    a1_src = np.asarray(a1_src); a1_dst = np.asarray(a1_dst)
    a2_src = np.asarray(a2_src); a2_dst = np.asarray(a2_dst)

    core_of = dst // N_PC                      # owner core per edge
    # per-core node permutation: sort own nodes by in-degree desc (pads last)
    perms = []          # perms[c][i] = original node id at permuted position i
    inv_pos = np.zeros(N, dtype=np.int64)      # node -> position within its core
    degs = np.bincount(dst, minlength=N)
    for c in range(NCORES):
        own = np.arange(c * N_PC, (c + 1) * N_PC)
        order = own[np.argsort(-degs[own], kind='stable')]
        perm = np.concatenate([order, np.full(N_PAD - N_PC, -1, dtype=np.int64)])
        perms.append(perm)
        inv_pos[order] = np.arange(N_PC)

    # global table row of node n (pad rows belong to tail positions)
    def table_row(n):
        return (n // N_PC) * N_PAD + inv_pos[n]

    # per-core per-block caps (cross-core max) ------------------------------
    deg_mat = np.zeros((NCORES, N_PAD), dtype=np.int64)
    for c in range(NCORES):
        real = perms[c] >= 0
        deg_mat[c, real] = degs[perms[c][real]]
    caps = deg_mat.reshape(NCORES, NBLK, P).max(axis=2).max(axis=0)  # [NBLK]
    caps = np.maximum(caps, 1)
    slot_off = np.concatenate([[0], np.cumsum(caps)])                # col offsets
    tot_cols = int(slot_off[-1])

    # pad pair-row: last two table rows are core-7 pad nodes (deg 0, x rows 0)
    PAD_PAIR = (N_TOT - 2) // 2

    # per-core edge slot assignment -----------------------------------------
    idx_all = np.full((NCORES, P, tot_cols), PAD_PAIR, dtype=np.int64)
    m_all = np.zeros((NCORES, P, tot_cols), dtype=np.float32)
    tr_src = table_row(src)
    pos_in_core = inv_pos[dst]                 # permuted position of dst
    for c in range(NCORES):
        sel = core_of == c
        s_rows = tr_src[sel]
        d_pos = pos_in_core[sel]
        order = np.argsort(d_pos, kind='stable')
        s_rows = s_rows[order]; d_pos = d_pos[order]
        # j-th edge of each dst
        jj = np.arange(len(d_pos)) - np.searchsorted(d_pos, d_pos, side='left')
        b = d_pos // P; pp = d_pos % P
        cols = slot_off[b] + jj
        assert (jj < caps[b]).all()
        idx_all[c, pp, cols] = s_rows // 2
        m_all[c, pp, cols] = (s_rows % 2).astype(np.float32)

    # wrapped int16 index layout per block: flat order i=(col*128+p),
    # reshape(-1,16).T, tiled 8x over partitions
    idx_wrapped = np.zeros((NCORES, P, tot_cols * 8), dtype=np.int16)
    for c in range(NCORES):
        for b in range(NBLK):
            o0, o1 = slot_off[b], slot_off[b + 1]
            flat = idx_all[c][:, o0:o1].T.reshape(-1)      # (col, p) order
            wr = np.tile(flat.reshape(-1, 16).T, (8, 1)).astype(np.int16)
            idx_wrapped[c][:, o0 * 8:o1 * 8] = wr

    # weights ---------------------------------------------------------------
    W1cat = W1.transpose(1, 0, 2).reshape(F_IN, OUT1)      # [512, 64]
    ws1 = np.einsum('hfo,ho->fh', W1, a1_src)              # [512, 8]
    wd1 = np.einsum('hfo,ho->fh', W1, a1_dst)              # [512, 8]
    W1full = np.concatenate([W1cat, ws1, wd1], axis=1)     # [512, 80]
    W2cat = W2.transpose(1, 0, 2).reshape(OUT1, NLAB)      # [64, 64]
    ws2 = np.einsum('hfo,ho->fh', W2, a2_src)              # [64, 1]
    wd2 = np.einsum('hfo,ho->fh', W2, a2_dst)              # [64, 1]
    W2full = np.concatenate([W2cat, ws2, wd2], axis=1)     # [64, 66]

    # per-core transposed inputs (permuted, padded) -------------------------
    xT = np.zeros((NCORES, F_IN, N_PAD), dtype=np.float32)
    for c in range(NCORES):
        real = perms[c] >= 0
        xT[c][:, real] = inputs[perms[c][real]].T

    meta = dict(caps=caps, slot_off=slot_off, tot_cols=tot_cols, perms=perms)
    mcat = np.stack([1.0 - m_all, m_all], axis=-1)      # [NC, P, TC, 2]
    mcat = mcat.reshape(NCORES, P, tot_cols * 2)
    per_core = dict(
        xT=[_bf16(xT[c]) for c in range(NCORES)],
        idx=[idx_wrapped[c] for c in range(NCORES)],
        m=[_bf16(mcat[c]) for c in range(NCORES)],
    )
    shared = dict(W1full=_bf16(W1full), W2full=_bf16(W2full))
    return meta, per_core, shared


def build_kernel(meta):
    import concourse.bass as bass
    import concourse.bacc as bacc
    import concourse.tile as tile
    from concourse import mybir
    from concourse.masks import make_identity

    bf16 = mybir.dt.bfloat16; f32 = mybir.dt.float32; i16 = mybir.dt.int16
    AL = mybir.AluOpType; AF = mybir.ActivationFunctionType; AX = mybir.AxisListType

    caps = [int(x) for x in meta['caps']]
    slot_off = [int(x) for x in meta['slot_off']]
    TC = int(meta['tot_cols'])

    nc = bacc.Bacc("TRN2", target_bir_lowering=False, debug=False,
                   enable_asserts=True, num_devices=NCORES)

    t_xT = nc.dram_tensor("xT", [F_IN, N_PAD], bf16, kind="ExternalInput").ap()
    t_idx = nc.dram_tensor("idx", [P, TC * 8], i16, kind="ExternalInput").ap()
    t_m = nc.dram_tensor("m", [P, TC * 2], bf16, kind="ExternalInput").ap()
    t_W1 = nc.dram_tensor("W1full", [F_IN, 80], bf16, kind="ExternalInput").ap()
    t_W2 = nc.dram_tensor("W2full", [OUT1, 66], bf16, kind="ExternalInput").ap()
    t_out = nc.dram_tensor("out", [N_PAD, NLAB], f32, kind="ExternalOutput").ap()

    KCH = F_IN // P  # 4 k-chunks

    with tile.TileContext(nc) as tc:
        with tc.tile_pool(name="dram", bufs=1, space="DRAM") as dram, \
             tc.tile_pool(name="const", bufs=1) as cpool, \
             tc.tile_pool(name="work", bufs=3) as wpool, \
             tc.tile_pool(name="gath", bufs=3) as gpool, \
             tc.tile_pool(name="msgsp", bufs=2) as mpool, \
             tc.tile_pool(name="psum", bufs=2, space="PSUM") as pp, \
             tc.tile_pool(name="psum1", bufs=2, space="PSUM") as pp1:

            ident = cpool.tile([P, P], f32)
            make_identity(nc, ident[:])

            w1_sb = cpool.tile([P, KCH, 80], bf16)
            nc.sync.dma_start(out=w1_sb[:], in_=t_W1.rearrange("(k p) w -> p k w", p=P))
            w2_sb = cpool.tile([OUT1, 66], bf16)
            nc.sync.dma_start(out=w2_sb[:], in_=t_W2[:])
            m_sb = cpool.tile([P, TC, 2], bf16)
            nc.sync.dma_start(out=m_sb[:], in_=t_m[:].rearrange("p (c t) -> p c t", t=2))
            idx_sb = cpool.tile([P, TC * 8], i16)
            nc.sync.dma_start(out=idx_sb[:], in_=t_idx[:])

            ed1_all = cpool.tile([P, NBLK, H1], f32)
            ed2_all = cpool.tile([P, NBLK, 1], f32)

            # DRAM tables
            T1_loc = dram.tile([N_PAD, ROW], bf16)
            T2_loc = dram.tile([N_PAD, ROW], bf16)
            T1_full = dram.tile([N_TOT, ROW], bf16, addr_space="Shared")
            T2_full = dram.tile([N_TOT, ROW], bf16, addr_space="Shared")

            # ---------------- phase 1: transform layer 1 ----------------
            for b in range(NBLK):
                ps = pp.tile([P, 80], f32, tag="tf1", space="PSUM")
                for k in range(KCH):
                    xt = wpool.tile([P, P], bf16, tag="xt", bufs=4)
                    nc.sync.dma_start(out=xt[:],
                                      in_=t_xT[k * P:(k + 1) * P, b * P:(b + 1) * P])
                    nc.tensor.matmul(out=ps[:], lhsT=xt[:],
                                     rhs=w1_sb[:, k, :], start=(k == 0), stop=(k == KCH - 1))
                row = wpool.tile([P, ROW], bf16, tag="trow")
                nc.vector.tensor_copy(out=row[:, 0:72], in_=ps[:, 0:72])
                if b == NBLK - 1:
                    # pad nodes (tail partitions) must have es = -1e30
                    npad = N_PAD - N_PC  # 22
                    nc.gpsimd.affine_select(
                        out=row[:, ES_OFF:ES_OFF + H1], in_=row[:, ES_OFF:ES_OFF + H1],
                        pattern=[[0, H1]], compare_op=mybir.AluOpType.is_ge,
                        fill=NEG_BIG, base=P - npad - 1, channel_multiplier=-1)
                nc.sync.dma_start(out=T1_loc[b * P:(b + 1) * P, :], in_=row[:])
                nc.vector.tensor_copy(out=ed1_all[:, b, :], in_=ps[:, 72:80])

            # ---------------- all-gather table 1 ----------------
            nc.gpsimd.collective_compute(
                "AllGather", mybir.AluOpType.bypass,
                replica_groups=[list(range(NCORES))],
                ins=[T1_loc[:].opt()], outs=[T1_full[:].opt()])

            # pair view of the full table: [NPAIR, 256]
            T1_pair = T1_full[:].rearrange("(q t) r -> q (t r)", t=2)
            T2_pair = T2_full[:].rearrange("(q t) r -> q (t r)", t=2)

            def edge_block(b, T_pair, ed_ap, H, layer):
                """Process dst-block b for one layer. Returns (numer, den) tiles."""
                cap = caps[b]; o0 = slot_off[b]
                ni = cap * P
                g = gpool.tile([P, caps[0], 256], bf16, tag="g")
                n_g = min((ni + MAX_GATHER - 1) // MAX_GATHER, 8)
                step = ((cap + n_g - 1) // n_g)
                c0 = 0
                while c0 < cap:
                    c1 = min(c0 + step, cap)
                    nc.gpsimd.dma_gather(
                        out_ap=g[:, c0:c1, :], in_ap=T_pair,
                        idxs_ap=idx_sb[:, (o0 + c0) * 8:(o0 + c1) * 8],
                        num_idxs=(c1 - c0) * P, num_idxs_reg=(c1 - c0) * P,
                        elem_size=256, single_packet=False)
                    c0 = c1
                mm = m_sb[:, o0:o0 + cap, :]      # [P, cap, 2] = [1-m | m]

                # both-halves attention: s = es{A,B} + ed; e = max(s, 0.2*s);
                # ex = exp(e); exps = ex * [1-m | m]
                es2 = g[:, 0:cap, :].rearrange("p j (t r) -> p j t r", t=2)[:, :, :, ES_OFF:ES_OFF + H]
                s = wpool.tile([P, caps[0], 2, H1], f32, tag="s", name="st")[:, 0:cap, :, 0:H]
                nc.vector.tensor_tensor(
                    out=s, in0=es2,
                    in1=ed_ap.unsqueeze(1).unsqueeze(1).broadcast_to([P, cap, 2, H]),
                    op=AL.add)
                e = wpool.tile([P, caps[0], 2, H1], f32, tag="e", name="et")[:, 0:cap, :, 0:H]
                nc.vector.scalar_tensor_tensor(out=e, in0=s, scalar=LRELU_SLOPE,
                                               in1=s, op0=AL.mult, op1=AL.max)
                ex = wpool.tile([P, caps[0], 2, H1], f32, tag="exx", name="exxt")[:, 0:cap, :, 0:H]
                nc.scalar.activation(out=ex, in_=e, func=AF.Exp)
                exps = wpool.tile([P, caps[0], 2, H1], bf16, tag="exps", name="expst")[:, 0:cap, :, 0:H]
                nc.vector.tensor_tensor(
                    out=exps, in0=ex,
                    in1=mm.unsqueeze(-1).broadcast_to([P, cap, 2, H]), op=AL.mult)

                # msgs[p, j, half, h, o] = Wh[p, j, half, h, o] * exps[p, j, half, h]
                OUTD = 64
                msgs = mpool.tile([P, caps[0], 2, OUTD], bf16, tag="msgs", name="msgst")[:, 0:cap, :, :]
                wh = g[:, 0:cap, :].rearrange("p j (t r) -> p j t r", t=2)[:, :, :, 0:OUTD]
                if H > 1:
                    wh5 = wh.rearrange("p j t (h o) -> p j t h o", h=H)
                    ex5 = exps.unsqueeze(-1).broadcast_to([P, cap, 2, H, OUTD // H])
                    nc.vector.tensor_tensor(out=msgs.rearrange("p j t (h o) -> p j t h o", h=H),
                                            in0=wh5, in1=ex5, op=AL.mult)
                else:
                    ex4 = exps.broadcast_to([P, cap, 2, OUTD])
                    nc.vector.tensor_tensor(out=msgs, in0=wh, in1=ex4, op=AL.mult)

                # tree-reduce over (j, half) with contiguous chunk adds
                tre = mpool.tile([P, caps[0], OUTD], f32, tag="tree", name="treet")
                nc.vector.tensor_tensor(out=tre[:, 0:cap, :], in0=msgs[:, :, 0, :],
                                        in1=msgs[:, :, 1, :], op=AL.add)
                cur = cap
                while cur > 1:
                    mhalf = cur // 2
                    nc.vector.tensor_tensor(out=tre[:, 0:mhalf, :], in0=tre[:, 0:mhalf, :],
                                            in1=tre[:, cur - mhalf:cur, :], op=AL.add)
                    cur -= mhalf
                numer = tre[:, 0, :]
                den = wpool.tile([P, H1], f32, tag="den", name="dent")[:, 0:H]
                nc.vector.tensor_reduce(out=den, in_=exps.rearrange("p j t h -> p h (j t)"),
                                        axis=AX.X, op=AL.add)
                nc.vector.tensor_scalar_add(out=den, in0=den, scalar1=1e-10)
                rec = wpool.tile([P, H1], f32, tag="rec", name="rect")[:, 0:H]
                nc.vector.reciprocal(out=rec, in_=den)
                hpre = wpool.tile([P, OUTD], f32, tag="hpre")
                if H > 1:
                    nc.vector.tensor_tensor(
                        out=hpre[:].rearrange("p (h o) -> p h o", h=H),
                        in0=numer.rearrange("p (h o) -> p h o", h=H),
                        in1=rec.unsqueeze(-1).broadcast_to([P, H, OUTD // H]), op=AL.mult)
                else:
                    nc.vector.tensor_tensor(out=hpre[:], in0=numer,
                                            in1=rec.broadcast_to([P, OUTD]), op=AL.mult)
                return hpre

            # ---------------- phase 2: layer-1 edges + layer-2 transform ----------------
            for b in range(NBLK):
                hpre = edge_block(b, T1_pair, ed1_all[:, b, :], H1, 1)
                # ELU: h = relu(x) + min(exp(x),1) - 1
                ex_h = wpool.tile([P, OUT1], f32, tag="eluex")
                nc.scalar.activation(out=ex_h[:], in_=hpre[:], func=AF.Exp)
                r_h = wpool.tile([P, OUT1], f32, tag="elur")
                nc.vector.tensor_scalar_max(out=r_h[:], in0=hpre[:], scalar1=0.0)
                h = wpool.tile([P, OUT1], f32, tag="hfin")
                nc.vector.scalar_tensor_tensor(out=h[:], in0=ex_h[:], scalar=1.0,
                                               in1=r_h[:], op0=AL.min, op1=AL.add)
                nc.vector.tensor_scalar_add(out=h[:], in0=h[:], scalar1=-1.0)
                # transpose h -> [64, 128]
                hT_ps = pp1.tile([OUT1, P], f32, tag="hT", space="PSUM")
                nc.tensor.transpose(out=hT_ps[:], in_=h[:], identity=ident[:])
                hT = wpool.tile([OUT1, P], bf16, tag="hTb")
                nc.vector.tensor_copy(out=hT[:], in_=hT_ps[:])
                # layer-2 transform
                ps2 = pp.tile([P, 66], f32, tag="tf2", space="PSUM")
                nc.tensor.matmul(out=ps2[:], lhsT=hT[:], rhs=w2_sb[:], start=True, stop=True)
                row2 = wpool.tile([P, ROW], bf16, tag="trow")
                nc.vector.tensor_copy(out=row2[:, 0:65], in_=ps2[:, 0:65])
                if b == NBLK - 1:
                    npad = N_PAD - N_PC
                    nc.gpsimd.affine_select(
                        out=row2[:, ES_OFF:ES_OFF + 1], in_=row2[:, ES_OFF:ES_OFF + 1],
                        pattern=[[0, 1]], compare_op=mybir.AluOpType.is_ge,
                        fill=NEG_BIG, base=P - npad - 1, channel_multiplier=-1)
                nc.sync.dma_start(out=T2_loc[b * P:(b + 1) * P, :], in_=row2[:])
                nc.vector.tensor_copy(out=ed2_all[:, b, :], in_=ps2[:, 65:66])

            # ---------------- all-gather table 2 ----------------
            nc.gpsimd.collective_compute(
                "AllGather", mybir.AluOpType.bypass,
                replica_groups=[list(range(NCORES))],
                ins=[T2_loc[:].opt()], outs=[T2_full[:].opt()])

            # ---------------- phase 3: layer-2 edges + softmax ----------------
            for b in range(NBLK):
                opre = edge_block(b, T2_pair, ed2_all[:, b, :], H2, 2)
                rm = wpool.tile([P, 1], f32, tag="rm")
                nc.vector.tensor_reduce(out=rm[:], in_=opre[:], axis=AX.X,
                                        op=AL.max, negate=True)
                z = wpool.tile([P, NLAB], f32, tag="z")
                zsum = wpool.tile([P, 1], f32, tag="zsum")
                nc.scalar.activation(out=z[:], in_=opre[:], func=AF.Exp,
                                     bias=rm[:], accum_out=zsum[:])
                recs = wpool.tile([P, 1], f32, tag="recs")
                nc.vector.reciprocal(out=recs[:], in_=zsum[:])
                fin = wpool.tile([P, NLAB], f32, tag="fin")
                nc.vector.tensor_tensor(out=fin[:], in0=z[:],
                                        in1=recs[:].broadcast_to([P, NLAB]), op=AL.mult)
                nc.sync.dma_start(out=t_out[b * P:(b + 1) * P, :], in_=fin[:])

    nc.compile()
    return nc


def _install_ntff_shim():
    """antenv.axon_hooks is absent in this image; register the NTFF profile
    hook so trace=True can capture exec times. No-op if already present."""
    import types
    try:
        import antenv.axon_hooks  # noqa: F401
        return
    except ImportError:
        pass
    try:
        import antenv
        from trn_agent_boot.trn_boot import _ntff_profile_via_ctypes
        mod = types.ModuleType("antenv.axon_hooks")
        mod._hook = _ntff_profile_via_ctypes('/opt/axon/libaxon_pjrt.so')
        mod.set_axon_ntff_profile_hook = lambda h: setattr(mod, "_hook", h)
        mod.get_axon_ntff_profile_hook = lambda: mod._hook
        sys.modules["antenv.axon_hooks"] = mod
        antenv.axon_hooks = mod
    except Exception:
        pass


def kernel(inputs, W1, a1_src, a1_dst, W2, a2_src, a2_dst, src, dst):
    from concourse import bass_utils
    if int(os.environ.get("GAT_TRACE", "0")):
        _install_ntff_shim()
    meta, per_core, shared = host_prep(inputs, W1, a1_src, a1_dst, W2,
                                       a2_src, a2_dst, src, dst)
    nc = build_kernel(meta)
    in_maps = []
    for c in range(NCORES):
        in_maps.append(dict(
            xT=per_core['xT'][c], idx=per_core['idx'][c], m=per_core['m'][c],
            W1full=shared['W1full'], W2full=shared['W2full']))
    res = bass_utils.run_bass_kernel_spmd(
        nc, in_maps, core_ids=list(range(NCORES)),
        trace=bool(int(os.environ.get("GAT_TRACE", "0"))),
        trace_cores=list(range(NCORES)) if int(os.environ.get("GAT_TRACE", "0")) else None)
    kernel.last_exec_time_ns = res.exec_time_ns
    out = np.zeros((N, NLAB), dtype=np.float32)
    for c in range(NCORES):
        o = res.results[c]["out"]
        perm = meta['perms'][c]
        real = perm >= 0
        out[perm[real]] = o[real]
    return out


def mirror(inputs, W1, a1_src, a1_dst, W2, a2_src, a2_dst, src, dst):
    """Numpy mirror of the kernel's exact dataflow (incl. bf16 rounding of
    tables) for layout validation without hardware."""
    meta, per_core, shared = host_prep(inputs, W1, a1_src, a1_dst, W2,
                                       a2_src, a2_dst, src, dst)
    caps = meta['caps']; slot_off = meta['slot_off']; TC = meta['tot_cols']
    W1full = shared['W1full'].astype(np.float32)
    W2full = shared['W2full'].astype(np.float32)
    out = np.zeros((N, NLAB), dtype=np.float32)

    # build tables per core, then allgather
    T1 = np.zeros((N_TOT, ROW), dtype=np.float32)
    ed1 = np.zeros((NCORES, N_PAD, H1), dtype=np.float32)
    for c in range(NCORES):
        xT = per_core['xT'][c].astype(np.float32)
        t = xT.T @ W1full                       # [N_PAD, 80]
        rows = np.zeros((N_PAD, ROW), np.float32)
        rows[:, 0:72] = t[:, 0:72]
        rows[N_PC:, ES_OFF:ES_OFF + H1] = NEG_BIG
        T1[c * N_PAD:(c + 1) * N_PAD] = _bf16(rows).astype(np.float32)
        ed1[c] = t[:, 72:80]

    def edge_phase(c, T, ed, H):
        Tp = T.reshape(NPAIR, 256)
        idx = per_core['idx'][c]
        m = per_core['m'][c].astype(np.float32)
        res = np.zeros((N_PAD, OUT1), np.float32)
        for b in range(NBLK):
            cap = caps[b]; o0 = slot_off[b]
            # unwrap idx: stored wrapped per block
            wr = idx[:16, o0 * 8:(o0 + cap) * 8]
            flat = wr.T.reshape(-1)             # undo .reshape(-1,16).T
            g = Tp[flat.astype(np.int64)].reshape(cap, P, 256).transpose(1, 0, 2)
            mm = m[:, o0:o0 + cap]
            esA = g[:, :, ES_OFF:ES_OFF + H]
            esB = g[:, :, 128 + ES_OFF:128 + ES_OFF + H]
            es = esA + mm[:, :, None] * (esB - esA)
            s = es + ed[b * P:(b + 1) * P].reshape(P, 1, H)
            e = np.where(s > 0, s, LRELU_SLOPE * s)
            ex = np.exp(e)
            exB = ex * mm[:, :, None]; exA = ex - exB
            whA = g[:, :, 0:64]; whB = g[:, :, 128:192]
            if H > 1:
                o = OUT1 // H
                msA = whA.reshape(P, cap, H, o) * exA[:, :, :, None]
                msB = whB.reshape(P, cap, H, o) * exB[:, :, :, None]
                numer = (msA + msB).sum(axis=1).reshape(P, OUT1)
                den = (exA + exB).sum(axis=1)
                hpre = (numer.reshape(P, H, o) / (den[:, :, None] + 1e-10)).reshape(P, OUT1)
            else:
                msA = whA * exA; msB = whB * exB
                numer = (msA + msB).sum(axis=1)
                den = (exA + exB).sum(axis=1)
                hpre = numer / (den + 1e-10)
            res[b * P:(b + 1) * P] = hpre
        return res

    T2 = np.zeros((N_TOT, ROW), dtype=np.float32)
    ed2 = np.zeros((NCORES, N_PAD, 1), dtype=np.float32)
    h_all = {}
    for c in range(NCORES):
        hpre = edge_phase(c, T1, ed1[c], H1)
        h = np.maximum(hpre, 0) + np.minimum(np.exp(hpre), 1.0) - 1.0
        h_all[c] = h
        t2 = _bf16(h).astype(np.float32) @ W2full
        rows = np.zeros((N_PAD, ROW), np.float32)
        rows[:, 0:65] = t2[:, 0:65]
        rows[N_PC:, ES_OFF:ES_OFF + 1] = NEG_BIG
        T2[c * N_PAD:(c + 1) * N_PAD] = _bf16(rows).astype(np.float32)
        ed2[c] = t2[:, 65:66]

    for c in range(NCORES):
        opre = edge_phase(c, T2, ed2[c], H2)[:, 0:NLAB]
        z = np.exp(opre - opre.max(axis=1, keepdims=True))
        fin = z / z.sum(axis=1, keepdims=True)
        perm = meta['perms'][c]; real = perm >= 0
        out[perm[real]] = fin[real]
    return out


# revision 10
# speedup vs baseline: 1.2465x; 1.0316x over previous
"""GAT 2-layer kernel for 8 Trainium2 NeuronCores (Bass/Tile).

Strategy (1D partition by dst):
- Each core owns N/8 dst nodes (padded to blocks of 128). Host permutes each
  core's nodes so in-degrees are sorted descending -> per-block edge-slot
  capacity cap_b (= cross-core max block degree) stays tight.
- Transform phase: per node-block matmul x @ [W | W@a_src | W@a_dst] gives
  [Wh | es | ed] in one pass. [Wh|es] rows (bf16, 128 elems = 256B) form the
  gather table; ed stays on-chip (per-partition, dst-local).
- AllGather the per-core tables -> every core holds the full [N, 128] bf16
  table in DRAM.
- Edge phase: per dst-block one dma_gather with PAIR rows (elem 256 bf16 =
  512B covering nodes 2k/2k+1, idx = src>>1 fits int16); edge slot (p, j) =
  j-th in-edge of the block's p-th dst. Half-select is folded into the
  attention scaling (exA = ex*(1-m), exB = ex*m).
- Attention: e = lrelu(es_sel + ed), ex = exp(e) (max-subtraction is skipped:
  |e| is bounded by a few units so exp cannot overflow; softmax is shift
  invariant). numer = sum_j ex*Wh via one multiply + one strided reduce;
  den = sum_j ex. Pad slots point at a pad pair-row whose es = -1e30 -> ex=0.
- Layer 2 reuses the same edge slots/indices; final row softmax on chip.
"""

import sys, os
sys.path.insert(0, '/opt/trn_rl_repo')

import numpy as np

# ---- problem constants (from the reference; hardcoded, not read from disk) ----
N = 50000
E = 800000
F_IN = 512
H1 = 8
F_HID = 8
OUT1 = H1 * F_HID          # 64
H2 = 1
NLAB = 64
LRELU_SLOPE = 0.2
NCORES = 8
P = 128

N_PC = N // NCORES         # 6250
NBLK = (N_PC + P - 1) // P # 49
N_PAD = NBLK * P           # 6272
N_TOT = N_PAD * NCORES     # 50176
NPAIR = N_TOT // 2         # 25088
ROW = 128                  # table row elems (bf16): [Wh 64 | es 8 | pad]
ES_OFF = 64
NEG_BIG = -1.0e30
MAX_GATHER = 8192          # HW limit for one dma_gather


def _bf16(x):
    import jax.numpy as jnp
    return np.asarray(jnp.asarray(np.asarray(x), dtype=jnp.bfloat16))


def host_prep(inputs, W1, a1_src, a1_dst, W2, a2_src, a2_dst, src, dst):
    """Pure-numpy preparation of all per-core tensors + layout metadata."""
    inputs = np.asarray(inputs); src = np.asarray(src); dst = np.asarray(dst)
    W1 = np.asarray(W1); W2 = np.asarray(W2)
    a1_src = np.asarray(a1_src); a1_dst = np.asarray(a1_dst)
    a2_src = np.asarray(a2_src); a2_dst = np.asarray(a2_dst)

    core_of = dst // N_PC                      # owner core per edge
    # per-core node permutation: sort own nodes by in-degree desc (pads last)
    perms = []          # perms[c][i] = original node id at permuted position i
    inv_pos = np.zeros(N, dtype=np.int64)      # node -> position within its core
    degs = np.bincount(dst, minlength=N)
    for c in range(NCORES):
        own = np.arange(c * N_PC, (c + 1) * N_PC)
        order = own[np.argsort(-degs[own], kind='stable')]
        perm = np.concatenate([order, np.full(N_PAD - N_PC, -1, dtype=np.int64)])
        perms.append(perm)
        inv_pos[order] = np.arange(N_PC)

    # global table row of node n (pad rows belong to tail positions)
    def table_row(n):
        return (n // N_PC) * N_PAD + inv_pos[n]

    # per-core per-block caps (cross-core max) ------------------------------
    deg_mat = np.zeros((NCORES, N_PAD), dtype=np.int64)
    for c in range(NCORES):
        real = perms[c] >= 0
        deg_mat[c, real] = degs[perms[c][real]]
    caps = deg_mat.reshape(NCORES, NBLK, P).max(axis=2).max(axis=0)  # [NBLK]
    caps = np.maximum(caps, 1)
    slot_off = np.concatenate([[0], np.cumsum(caps)])                # col offsets
    tot_cols = int(slot_off[-1])

    # pad pair-row: last two table rows are core-7 pad nodes (deg 0, x rows 0)
    PAD_PAIR = (N_TOT - 2) // 2

    # per-core edge slot assignment -----------------------------------------
    idx_all = np.full((NCORES, P, tot_cols), PAD_PAIR, dtype=np.int64)
    m_all = np.zeros((NCORES, P, tot_cols), dtype=np.float32)
    tr_src = table_row(src)
    pos_in_core = inv_pos[dst]                 # permuted position of dst
    for c in range(NCORES):
        sel = core_of == c
        s_rows = tr_src[sel]
        d_pos = pos_in_core[sel]
        order = np.argsort(d_pos, kind='stable')
        s_rows = s_rows[order]; d_pos = d_pos[order]
        # j-th edge of each dst
        jj = np.arange(len(d_pos)) - np.searchsorted(d_pos, d_pos, side='left')
        b = d_pos // P; pp = d_pos % P
        cols = slot_off[b] + jj
        assert (jj < caps[b]).all()
        idx_all[c, pp, cols] = s_rows // 2
        m_all[c, pp, cols] = (s_rows % 2).astype(np.float32)

    # wrapped int16 index layout per block: flat order i=(col*128+p),
    # reshape(-1,16).T, tiled 8x over partitions
    idx_wrapped = np.zeros((NCORES, P, tot_cols * 8), dtype=np.int16)
    for c in range(NCORES):
        for b in range(NBLK):
            o0, o1 = slot_off[b], slot_off[b + 1]
            flat = idx_all[c][:, o0:o1].T.reshape(-1)      # (col, p) order
            wr = np.tile(flat.reshape(-1, 16).T, (8, 1)).astype(np.int16)
            idx_wrapped[c][:, o0 * 8:o1 * 8] = wr

    # weights ---------------------------------------------------------------
    W1cat = W1.transpose(1, 0, 2).reshape(F_IN, OUT1)      # [512, 64]
    ws1 = np.einsum('hfo,ho->fh', W1, a1_src)              # [512, 8]
    wd1 = np.einsum('hfo,ho->fh', W1, a1_dst)              # [512, 8]
    W1full = np.concatenate([W1cat, ws1, wd1], axis=1)     # [512, 80]
    W2cat = W2.transpose(1, 0, 2).reshape(OUT1, NLAB)      # [64, 64]
    ws2 = np.einsum('hfo,ho->fh', W2, a2_src)              # [64, 1]
    wd2 = np.einsum('hfo,ho->fh', W2, a2_dst)              # [64, 1]
    W2full = np.concatenate([W2cat, ws2, wd2], axis=1)     # [64, 66]

    # per-core transposed inputs (permuted, padded) -------------------------
    xT = np.zeros((NCORES, F_IN, N_PAD), dtype=np.float32)
    for c in range(NCORES):
        real = perms[c] >= 0
        xT[c][:, real] = inputs[perms[c][real]].T

    meta = dict(caps=caps, slot_off=slot_off, tot_cols=tot_cols, perms=perms)
    mcat = np.stack([1.0 - m_all, m_all], axis=-1)      # [NC, P, TC, 2]
    mcat = mcat.reshape(NCORES, P, tot_cols * 2)
    per_core = dict(
        xT=[_bf16(xT[c]) for c in range(NCORES)],
        idx=[idx_wrapped[c] for c in range(NCORES)],
        m=[_bf16(mcat[c]) for c in range(NCORES)],
    )
    shared = dict(W1full=_bf16(W1full), W2full=_bf16(W2full))
    return meta, per_core, shared


def build_kernel(meta):
    import concourse.bass as bass
    import concourse.bacc as bacc
    import concourse.tile as tile
    from concourse import mybir
    from concourse.masks import make_identity

    bf16 = mybir.dt.bfloat16; f32 = mybir.dt.float32; i16 = mybir.dt.int16
    AL = mybir.AluOpType; AF = mybir.ActivationFunctionType; AX = mybir.AxisListType

    caps = [int(x) for x in meta['caps']]
    slot_off = [int(x) for x in meta['slot_off']]
    TC = int(meta['tot_cols'])

    nc = bacc.Bacc("TRN2", target_bir_lowering=False, debug=False,
                   enable_asserts=True, num_devices=NCORES)

    t_xT = nc.dram_tensor("xT", [F_IN, N_PAD], bf16, kind="ExternalInput").ap()
    t_idx = nc.dram_tensor("idx", [P, TC * 8], i16, kind="ExternalInput").ap()
    t_m = nc.dram_tensor("m", [P, TC * 2], bf16, kind="ExternalInput").ap()
    t_W1 = nc.dram_tensor("W1full", [F_IN, 80], bf16, kind="ExternalInput").ap()
    t_W2 = nc.dram_tensor("W2full", [OUT1, 66], bf16, kind="ExternalInput").ap()
    t_out = nc.dram_tensor("out", [N_PAD, NLAB], f32, kind="ExternalOutput").ap()

    KCH = F_IN // P  # 4 k-chunks

    with tile.TileContext(nc) as tc:
        with tc.tile_pool(name="dram", bufs=1, space="DRAM") as dram, \
             tc.tile_pool(name="const", bufs=1) as cpool, \
             tc.tile_pool(name="work", bufs=4) as wpool, \
             tc.tile_pool(name="gath", bufs=3) as gpool, \
             tc.tile_pool(name="msgsp", bufs=3) as mpool, \
             tc.tile_pool(name="psum", bufs=2, space="PSUM") as pp, \
             tc.tile_pool(name="psum1", bufs=2, space="PSUM") as pp1:

            ident = cpool.tile([P, P], f32)
            make_identity(nc, ident[:])

            w1_sb = cpool.tile([P, KCH, 80], bf16)
            nc.sync.dma_start(out=w1_sb[:], in_=t_W1.rearrange("(k p) w -> p k w", p=P))
            w2_sb = cpool.tile([OUT1, 66], bf16)
            nc.sync.dma_start(out=w2_sb[:], in_=t_W2[:])
            m_sb = cpool.tile([P, TC, 2], bf16)
            nc.sync.dma_start(out=m_sb[:], in_=t_m[:].rearrange("p (c t) -> p c t", t=2))
            idx_sb = cpool.tile([P, TC * 8], i16)
            nc.sync.dma_start(out=idx_sb[:], in_=t_idx[:])

            ed1_all = cpool.tile([P, NBLK, H1], f32)
            ed2_all = cpool.tile([P, NBLK, 1], f32)

            # DRAM tables
            T1_loc = dram.tile([N_PAD, ROW], bf16)
            T2_loc = dram.tile([N_PAD, ROW], bf16)
            T1_full = dram.tile([N_TOT, ROW], bf16, addr_space="Shared")
            T2_full = dram.tile([N_TOT, ROW], bf16, addr_space="Shared")

            # ---------------- phase 1: transform layer 1 ----------------
            for b in range(NBLK):
                ps = pp.tile([P, 80], f32, tag="tf1", space="PSUM")
                for kk in range(KCH // 2):
                    xt = wpool.tile([P, 2, P], bf16, tag="xt", bufs=4)
                    nc.sync.dma_start(
                        out=xt[:],
                        in_=t_xT[kk * 2 * P:(kk + 1) * 2 * P, b * P:(b + 1) * P]
                            .rearrange("(k p) n -> p k n", k=2))
                    for k2 in range(2):
                        k = kk * 2 + k2
                        nc.tensor.matmul(out=ps[:], lhsT=xt[:, k2, :],
                                         rhs=w1_sb[:, k, :], start=(k == 0), stop=(k == KCH - 1))
                row = wpool.tile([P, ROW], bf16, tag="trow")
                nc.vector.tensor_copy(out=row[:, 0:72], in_=ps[:, 0:72])
                if b == NBLK - 1:
                    # pad nodes (tail partitions) must have es = -1e30
                    npad = N_PAD - N_PC  # 22
                    nc.gpsimd.affine_select(
                        out=row[:, ES_OFF:ES_OFF + H1], in_=row[:, ES_OFF:ES_OFF + H1],
                        pattern=[[0, H1]], compare_op=mybir.AluOpType.is_ge,
                        fill=NEG_BIG, base=P - npad - 1, channel_multiplier=-1)
                nc.sync.dma_start(out=T1_loc[b * P:(b + 1) * P, :], in_=row[:])
                nc.vector.tensor_copy(out=ed1_all[:, b, :], in_=ps[:, 72:80])

            # ---------------- all-gather table 1 ----------------
            nc.gpsimd.collective_compute(
                "AllGather", mybir.AluOpType.bypass,
                replica_groups=[list(range(NCORES))],
                ins=[T1_loc[:].opt()], outs=[T1_full[:].opt()])

            # pair view of the full table: [NPAIR, 256]
            T1_pair = T1_full[:].rearrange("(q t) r -> q (t r)", t=2)
            T2_pair = T2_full[:].rearrange("(q t) r -> q (t r)", t=2)

            def edge_block(b, T_pair, ed_ap, H, layer):
                """Process dst-block b for one layer. Returns (numer, den) tiles."""
                cap = caps[b]; o0 = slot_off[b]
                ni = cap * P
                g = gpool.tile([P, caps[0], 256], bf16, tag="g")
                n_g = min((ni + MAX_GATHER - 1) // MAX_GATHER, 8)
                step = ((cap + n_g - 1) // n_g)
                c0 = 0
                while c0 < cap:
                    c1 = min(c0 + step, cap)
                    nc.gpsimd.dma_gather(
                        out_ap=g[:, c0:c1, :], in_ap=T_pair,
                        idxs_ap=idx_sb[:, (o0 + c0) * 8:(o0 + c1) * 8],
                        num_idxs=(c1 - c0) * P, num_idxs_reg=(c1 - c0) * P,
                        elem_size=256, single_packet=False)
                    c0 = c1
                mm = m_sb[:, o0:o0 + cap, :]      # [P, cap, 2] = [1-m | m]

                # both-halves attention: s = es{A,B} + ed; e = max(s, 0.2*s);
                # ex = exp(e); exps = ex * [1-m | m]
                es2 = g[:, 0:cap, :].rearrange("p j (t r) -> p j t r", t=2)[:, :, :, ES_OFF:ES_OFF + H]
                s = wpool.tile([P, caps[0], 2, H1], f32, tag="s", name="st")[:, 0:cap, :, 0:H]
                nc.vector.tensor_tensor(
                    out=s, in0=es2,
                    in1=ed_ap.unsqueeze(1).unsqueeze(1).broadcast_to([P, cap, 2, H]),
                    op=AL.add)
                e = wpool.tile([P, caps[0], 2, H1], f32, tag="e", name="et")[:, 0:cap, :, 0:H]
                nc.vector.scalar_tensor_tensor(out=e, in0=s, scalar=LRELU_SLOPE,
                                               in1=s, op0=AL.mult, op1=AL.max)
                ex = wpool.tile([P, caps[0], 2, H1], f32, tag="exx", name="exxt")[:, 0:cap, :, 0:H]
                nc.scalar.activation(out=ex, in_=e, func=AF.Exp)
                exps = wpool.tile([P, caps[0], 2, H1], bf16, tag="exps", name="expst")[:, 0:cap, :, 0:H]
                nc.vector.tensor_tensor(
                    out=exps, in0=ex,
                    in1=mm.unsqueeze(-1).broadcast_to([P, cap, 2, H]), op=AL.mult)

                # msgs[p, j, half, h, o] = Wh[p, j, half, h, o] * exps[p, j, half, h]
                OUTD = 64
                msgs = mpool.tile([P, caps[0], 2, OUTD], bf16, tag="msgs", name="msgst")[:, 0:cap, :, :]
                wh = g[:, 0:cap, :].rearrange("p j (t r) -> p j t r", t=2)[:, :, :, 0:OUTD]
                if H > 1:
                    wh5 = wh.rearrange("p j t (h o) -> p j t h o", h=H)
                    ex5 = exps.unsqueeze(-1).broadcast_to([P, cap, 2, H, OUTD // H])
                    nc.vector.tensor_tensor(out=msgs.rearrange("p j t (h o) -> p j t h o", h=H),
                                            in0=wh5, in1=ex5, op=AL.mult)
                else:
                    ex4 = exps.broadcast_to([P, cap, 2, OUTD])
                    nc.vector.tensor_tensor(out=msgs, in0=wh, in1=ex4, op=AL.mult)

                # tree-reduce over (j, half) with contiguous chunk adds
                tre = mpool.tile([P, caps[0], OUTD], f32, tag="tree", name="treet")
                nc.vector.tensor_tensor(out=tre[:, 0:cap, :], in0=msgs[:, :, 0, :],
                                        in1=msgs[:, :, 1, :], op=AL.add)
                cur = cap
                while cur > 1:
                    mhalf = cur // 2
                    nc.vector.tensor_tensor(out=tre[:, 0:mhalf, :], in0=tre[:, 0:mhalf, :],
                                            in1=tre[:, cur - mhalf:cur, :], op=AL.add)
                    cur -= mhalf
                numer = tre[:, 0, :]
                den = wpool.tile([P, H1], f32, tag="den", name="dent")[:, 0:H]
                nc.vector.tensor_reduce(out=den, in_=exps.rearrange("p j t h -> p h (j t)"),
                                        axis=AX.X, op=AL.add)
                nc.vector.tensor_scalar_add(out=den, in0=den, scalar1=1e-10)
                rec = wpool.tile([P, H1], f32, tag="rec", name="rect")[:, 0:H]
                nc.vector.reciprocal(out=rec, in_=den)
                hpre = wpool.tile([P, OUTD], f32, tag="hpre")
                if H > 1:
                    nc.vector.tensor_tensor(
                        out=hpre[:].rearrange("p (h o) -> p h o", h=H),
                        in0=numer.rearrange("p (h o) -> p h o", h=H),
                        in1=rec.unsqueeze(-1).broadcast_to([P, H, OUTD // H]), op=AL.mult)
                else:
                    nc.vector.tensor_tensor(out=hpre[:], in0=numer,
                                            in1=rec.broadcast_to([P, OUTD]), op=AL.mult)
                return hpre

            # ---------------- phase 2: layer-1 edges + layer-2 transform ----------------
            for b in range(NBLK):
                hpre = edge_block(b, T1_pair, ed1_all[:, b, :], H1, 1)
                # ELU: h = relu(x) + min(exp(x),1) - 1
                ex_h = wpool.tile([P, OUT1], f32, tag="eluex")
                nc.scalar.activation(out=ex_h[:], in_=hpre[:], func=AF.Exp)
                r_h = wpool.tile([P, OUT1], f32, tag="elur")
                nc.vector.tensor_scalar_max(out=r_h[:], in0=hpre[:], scalar1=0.0)
                h = wpool.tile([P, OUT1], f32, tag="hfin")
                nc.vector.scalar_tensor_tensor(out=h[:], in0=ex_h[:], scalar=1.0,
                                               in1=r_h[:], op0=AL.min, op1=AL.add)
                nc.vector.tensor_scalar_add(out=h[:], in0=h[:], scalar1=-1.0)
                # transpose h -> [64, 128]
                hT_ps = pp1.tile([OUT1, P], f32, tag="hT", space="PSUM")
                nc.tensor.transpose(out=hT_ps[:], in_=h[:], identity=ident[:])
                hT = wpool.tile([OUT1, P], bf16, tag="hTb")
                nc.vector.tensor_copy(out=hT[:], in_=hT_ps[:])
                # layer-2 transform
                ps2 = pp.tile([P, 66], f32, tag="tf2", space="PSUM")
                nc.tensor.matmul(out=ps2[:], lhsT=hT[:], rhs=w2_sb[:], start=True, stop=True)
                row2 = wpool.tile([P, ROW], bf16, tag="trow")
                nc.vector.tensor_copy(out=row2[:, 0:65], in_=ps2[:, 0:65])
                if b == NBLK - 1:
                    npad = N_PAD - N_PC
                    nc.gpsimd.affine_select(
                        out=row2[:, ES_OFF:ES_OFF + 1], in_=row2[:, ES_OFF:ES_OFF + 1],
                        pattern=[[0, 1]], compare_op=mybir.AluOpType.is_ge,
                        fill=NEG_BIG, base=P - npad - 1, channel_multiplier=-1)
                nc.sync.dma_start(out=T2_loc[b * P:(b + 1) * P, :], in_=row2[:])
                nc.vector.tensor_copy(out=ed2_all[:, b, :], in_=ps2[:, 65:66])

            # ---------------- all-gather table 2 ----------------
            nc.gpsimd.collective_compute(
                "AllGather", mybir.AluOpType.bypass,
                replica_groups=[list(range(NCORES))],
                ins=[T2_loc[:].opt()], outs=[T2_full[:].opt()])

            # ---------------- phase 3: layer-2 edges + softmax ----------------
            for b in range(NBLK):
                opre = edge_block(b, T2_pair, ed2_all[:, b, :], H2, 2)
                rm = wpool.tile([P, 1], f32, tag="rm")
                nc.vector.tensor_reduce(out=rm[:], in_=opre[:], axis=AX.X,
                                        op=AL.max, negate=True)
                z = wpool.tile([P, NLAB], f32, tag="z")
                zsum = wpool.tile([P, 1], f32, tag="zsum")
                nc.scalar.activation(out=z[:], in_=opre[:], func=AF.Exp,
                                     bias=rm[:], accum_out=zsum[:])
                recs = wpool.tile([P, 1], f32, tag="recs")
                nc.vector.reciprocal(out=recs[:], in_=zsum[:])
                fin = wpool.tile([P, NLAB], f32, tag="fin")
                nc.vector.tensor_tensor(out=fin[:], in0=z[:],
                                        in1=recs[:].broadcast_to([P, NLAB]), op=AL.mult)
                nc.sync.dma_start(out=t_out[b * P:(b + 1) * P, :], in_=fin[:])

    nc.compile()
    return nc


def _install_ntff_shim():
    """antenv.axon_hooks is absent in this image; register the NTFF profile
    hook so trace=True can capture exec times. No-op if already present."""
    import types
    try:
        import antenv.axon_hooks  # noqa: F401
        return
    except ImportError:
        pass
    try:
        import antenv
        from trn_agent_boot.trn_boot import _ntff_profile_via_ctypes
        mod = types.ModuleType("antenv.axon_hooks")
        mod._hook = _ntff_profile_via_ctypes('/opt/axon/libaxon_pjrt.so')
        mod.set_axon_ntff_profile_hook = lambda h: setattr(mod, "_hook", h)
        mod.get_axon_ntff_profile_hook = lambda: mod._hook
        sys.modules["antenv.axon_hooks"] = mod
        antenv.axon_hooks = mod
    except Exception:
        pass


def kernel(inputs, W1, a1_src, a1_dst, W2, a2_src, a2_dst, src, dst):
    from concourse import bass_utils
    if int(os.environ.get("GAT_TRACE", "0")):
        _install_ntff_shim()
    meta, per_core, shared = host_prep(inputs, W1, a1_src, a1_dst, W2,
                                       a2_src, a2_dst, src, dst)
    nc = build_kernel(meta)
    in_maps = []
    for c in range(NCORES):
        in_maps.append(dict(
            xT=per_core['xT'][c], idx=per_core['idx'][c], m=per_core['m'][c],
            W1full=shared['W1full'], W2full=shared['W2full']))
    res = bass_utils.run_bass_kernel_spmd(
        nc, in_maps, core_ids=list(range(NCORES)),
        trace=bool(int(os.environ.get("GAT_TRACE", "0"))),
        trace_cores=list(range(NCORES)) if int(os.environ.get("GAT_TRACE", "0")) else None)
    kernel.last_exec_time_ns = res.exec_time_ns
    out = np.zeros((N, NLAB), dtype=np.float32)
    for c in range(NCORES):
        o = res.results[c]["out"]
        perm = meta['perms'][c]
        real = perm >= 0
        out[perm[real]] = o[real]
    return out


def mirror(inputs, W1, a1_src, a1_dst, W2, a2_src, a2_dst, src, dst):
    """Numpy mirror of the kernel's exact dataflow (incl. bf16 rounding of
    tables) for layout validation without hardware."""
    meta, per_core, shared = host_prep(inputs, W1, a1_src, a1_dst, W2,
                                       a2_src, a2_dst, src, dst)
    caps = meta['caps']; slot_off = meta['slot_off']; TC = meta['tot_cols']
    W1full = shared['W1full'].astype(np.float32)
    W2full = shared['W2full'].astype(np.float32)
    out = np.zeros((N, NLAB), dtype=np.float32)

    # build tables per core, then allgather
    T1 = np.zeros((N_TOT, ROW), dtype=np.float32)
    ed1 = np.zeros((NCORES, N_PAD, H1), dtype=np.float32)
    for c in range(NCORES):
        xT = per_core['xT'][c].astype(np.float32)
        t = xT.T @ W1full                       # [N_PAD, 80]
        rows = np.zeros((N_PAD, ROW), np.float32)
        rows[:, 0:72] = t[:, 0:72]
        rows[N_PC:, ES_OFF:ES_OFF + H1] = NEG_BIG
        T1[c * N_PAD:(c + 1) * N_PAD] = _bf16(rows).astype(np.float32)
        ed1[c] = t[:, 72:80]

    def edge_phase(c, T, ed, H):
        Tp = T.reshape(NPAIR, 256)
        idx = per_core['idx'][c]
        m = per_core['m'][c].astype(np.float32)
        res = np.zeros((N_PAD, OUT1), np.float32)
        for b in range(NBLK):
            cap = caps[b]; o0 = slot_off[b]
            # unwrap idx: stored wrapped per block
            wr = idx[:16, o0 * 8:(o0 + cap) * 8]
            flat = wr.T.reshape(-1)             # undo .reshape(-1,16).T
            g = Tp[flat.astype(np.int64)].reshape(cap, P, 256).transpose(1, 0, 2)
            mm = m[:, o0:o0 + cap]
            esA = g[:, :, ES_OFF:ES_OFF + H]
            esB = g[:, :, 128 + ES_OFF:128 + ES_OFF + H]
            es = esA + mm[:, :, None] * (esB - esA)
            s = es + ed[b * P:(b + 1) * P].reshape(P, 1, H)
            e = np.where(s > 0, s, LRELU_SLOPE * s)
            ex = np.exp(e)
            exB = ex * mm[:, :, None]; exA = ex - exB
            whA = g[:, :, 0:64]; whB = g[:, :, 128:192]
            if H > 1:
                o = OUT1 // H
                msA = whA.reshape(P, cap, H, o) * exA[:, :, :, None]
                msB = whB.reshape(P, cap, H, o) * exB[:, :, :, None]
                numer = (msA + msB).sum(axis=1).reshape(P, OUT1)
                den = (exA + exB).sum(axis=1)
                hpre = (numer.reshape(P, H, o) / (den[:, :, None] + 1e-10)).reshape(P, OUT1)
            else:
                msA = whA * exA; msB = whB * exB
                numer = (msA + msB).sum(axis=1)
                den = (exA + exB).sum(axis=1)
                hpre = numer / (den + 1e-10)
            res[b * P:(b + 1) * P] = hpre
        return res

    T2 = np.zeros((N_TOT, ROW), dtype=np.float32)
    ed2 = np.zeros((NCORES, N_PAD, 1), dtype=np.float32)
    h_all = {}
    for c in range(NCORES):
        hpre = edge_phase(c, T1, ed1[c], H1)
        h = np.maximum(hpre, 0) + np.minimum(np.exp(hpre), 1.0) - 1.0
        h_all[c] = h
        t2 = _bf16(h).astype(np.float32) @ W2full
        rows = np.zeros((N_PAD, ROW), np.float32)
        rows[:, 0:65] = t2[:, 0:65]
        rows[N_PC:, ES_OFF:ES_OFF + 1] = NEG_BIG
        T2[c * N_PAD:(c + 1) * N_PAD] = _bf16(rows).astype(np.float32)
        ed2[c] = t2[:, 65:66]

    for c in range(NCORES):
        opre = edge_phase(c, T2, ed2[c], H2)[:, 0:NLAB]
        z = np.exp(opre - opre.max(axis=1, keepdims=True))
        fin = z / z.sum(axis=1, keepdims=True)
        perm = meta['perms'][c]; real = perm >= 0
        out[perm[real]] = fin[real]
    return out


# revision 11
# speedup vs baseline: 1.3036x; 1.0458x over previous
"""GAT 2-layer kernel for 8 Trainium2 NeuronCores (Bass/Tile).

Strategy (1D partition by dst):
- Each core owns N/8 dst nodes (padded to blocks of 128). Host permutes each
  core's nodes so in-degrees are sorted descending -> per-block edge-slot
  capacity cap_b (= cross-core max block degree) stays tight.
- Transform phase: per node-block matmul x @ [W | W@a_src | W@a_dst] gives
  [Wh | es | ed] in one pass. [Wh|es] rows (bf16, 128 elems = 256B) form the
  gather table; ed stays on-chip (per-partition, dst-local).
- AllGather the per-core tables -> every core holds the full [N, 128] bf16
  table in DRAM.
- Edge phase: per dst-block one dma_gather with PAIR rows (elem 256 bf16 =
  512B covering nodes 2k/2k+1, idx = src>>1 fits int16); edge slot (p, j) =
  j-th in-edge of the block's p-th dst. Half-select is folded into the
  attention scaling (exA = ex*(1-m), exB = ex*m).
- Attention: e = lrelu(es_sel + ed), ex = exp(e) (max-subtraction is skipped:
  |e| is bounded by a few units so exp cannot overflow; softmax is shift
  invariant). numer = sum_j ex*Wh via one multiply + one strided reduce;
  den = sum_j ex. Pad slots point at a pad pair-row whose es = -1e30 -> ex=0.
- Layer 2 reuses the same edge slots/indices; final row softmax on chip.
"""

import sys, os
sys.path.insert(0, '/opt/trn_rl_repo')

import numpy as np

# ---- problem constants (from the reference; hardcoded, not read from disk) ----
N = 50000
E = 800000
F_IN = 512
H1 = 8
F_HID = 8
OUT1 = H1 * F_HID          # 64
H2 = 1
NLAB = 64
LRELU_SLOPE = 0.2
NCORES = 8
P = 128

N_PC = N // NCORES         # 6250
NBLK = (N_PC + P - 1) // P # 49
N_PAD = NBLK * P           # 6272
N_TOT = N_PAD * NCORES     # 50176
NPAIR = N_TOT // 2         # 25088
ROW = 128                  # table row elems (bf16): [Wh 64 | es 8 | pad]
ES_OFF = 64
NEG_BIG = -1.0e30
MAX_GATHER = 8192          # HW limit for one dma_gather


def _bf16(x):
    import jax.numpy as jnp
    return np.asarray(jnp.asarray(np.asarray(x), dtype=jnp.bfloat16))


def host_prep(inputs, W1, a1_src, a1_dst, W2, a2_src, a2_dst, src, dst):
    """Pure-numpy preparation of all per-core tensors + layout metadata."""
    inputs = np.asarray(inputs); src = np.asarray(src); dst = np.asarray(dst)
    W1 = np.asarray(W1); W2 = np.asarray(W2)
    a1_src = np.asarray(a1_src); a1_dst = np.asarray(a1_dst)
    a2_src = np.asarray(a2_src); a2_dst = np.asarray(a2_dst)

    core_of = dst // N_PC                      # owner core per edge
    # per-core node permutation: sort own nodes by in-degree desc (pads last)
    perms = []          # perms[c][i] = original node id at permuted position i
    inv_pos = np.zeros(N, dtype=np.int64)      # node -> position within its core
    degs = np.bincount(dst, minlength=N)
    for c in range(NCORES):
        own = np.arange(c * N_PC, (c + 1) * N_PC)
        order = own[np.argsort(-degs[own], kind='stable')]
        perm = np.concatenate([order, np.full(N_PAD - N_PC, -1, dtype=np.int64)])
        perms.append(perm)
        inv_pos[order] = np.arange(N_PC)

    # global table row of node n (pad rows belong to tail positions)
    def table_row(n):
        return (n // N_PC) * N_PAD + inv_pos[n]

    # per-core per-block caps (cross-core max) ------------------------------
    deg_mat = np.zeros((NCORES, N_PAD), dtype=np.int64)
    for c in range(NCORES):
        real = perms[c] >= 0
        deg_mat[c, real] = degs[perms[c][real]]
    caps = deg_mat.reshape(NCORES, NBLK, P).max(axis=2).max(axis=0)  # [NBLK]
    caps = np.maximum(caps, 1)
    slot_off = np.concatenate([[0], np.cumsum(caps)])                # col offsets
    tot_cols = int(slot_off[-1])

    # pad pair-row: last two table rows are core-7 pad nodes (deg 0, x rows 0)
    PAD_PAIR = (N_TOT - 2) // 2

    # per-core edge slot assignment -----------------------------------------
    idx_all = np.full((NCORES, P, tot_cols), PAD_PAIR, dtype=np.int64)
    m_all = np.zeros((NCORES, P, tot_cols), dtype=np.float32)
    tr_src = table_row(src)
    pos_in_core = inv_pos[dst]                 # permuted position of dst
    for c in range(NCORES):
        sel = core_of == c
        s_rows = tr_src[sel]
        d_pos = pos_in_core[sel]
        order = np.argsort(d_pos, kind='stable')
        s_rows = s_rows[order]; d_pos = d_pos[order]
        # j-th edge of each dst
        jj = np.arange(len(d_pos)) - np.searchsorted(d_pos, d_pos, side='left')
        b = d_pos // P; pp = d_pos % P
        cols = slot_off[b] + jj
        assert (jj < caps[b]).all()
        idx_all[c, pp, cols] = s_rows // 2
        m_all[c, pp, cols] = (s_rows % 2).astype(np.float32)

    # wrapped int16 index layout per block: flat order i=(col*128+p),
    # reshape(-1,16).T, tiled 8x over partitions
    idx_wrapped = np.zeros((NCORES, P, tot_cols * 8), dtype=np.int16)
    for c in range(NCORES):
        for b in range(NBLK):
            o0, o1 = slot_off[b], slot_off[b + 1]
            flat = idx_all[c][:, o0:o1].T.reshape(-1)      # (col, p) order
            wr = np.tile(flat.reshape(-1, 16).T, (8, 1)).astype(np.int16)
            idx_wrapped[c][:, o0 * 8:o1 * 8] = wr

    # weights ---------------------------------------------------------------
    W1cat = W1.transpose(1, 0, 2).reshape(F_IN, OUT1)      # [512, 64]
    ws1 = np.einsum('hfo,ho->fh', W1, a1_src)              # [512, 8]
    wd1 = np.einsum('hfo,ho->fh', W1, a1_dst)              # [512, 8]
    W1full = np.concatenate([W1cat, ws1, wd1], axis=1)     # [512, 80]
    W2cat = W2.transpose(1, 0, 2).reshape(OUT1, NLAB)      # [64, 64]
    ws2 = np.einsum('hfo,ho->fh', W2, a2_src)              # [64, 1]
    wd2 = np.einsum('hfo,ho->fh', W2, a2_dst)              # [64, 1]
    W2full = np.concatenate([W2cat, ws2, wd2], axis=1)     # [64, 66]

    # per-core transposed inputs (permuted, padded) -------------------------
    xT = np.zeros((NCORES, F_IN, N_PAD), dtype=np.float32)
    for c in range(NCORES):
        real = perms[c] >= 0
        xT[c][:, real] = inputs[perms[c][real]].T

    meta = dict(caps=caps, slot_off=slot_off, tot_cols=tot_cols, perms=perms)
    mcat = np.stack([1.0 - m_all, m_all], axis=-1)      # [NC, P, TC, 2]
    mcat = mcat.reshape(NCORES, P, tot_cols * 2)
    per_core = dict(
        xT=[_bf16(xT[c]) for c in range(NCORES)],
        idx=[idx_wrapped[c] for c in range(NCORES)],
        m=[_bf16(mcat[c]) for c in range(NCORES)],
    )
    shared = dict(W1full=_bf16(W1full), W2full=_bf16(W2full))
    return meta, per_core, shared


def build_kernel(meta):
    import concourse.bass as bass
    import concourse.bacc as bacc
    import concourse.tile as tile
    from concourse import mybir
    from concourse.masks import make_identity

    bf16 = mybir.dt.bfloat16; f32 = mybir.dt.float32; i16 = mybir.dt.int16
    AL = mybir.AluOpType; AF = mybir.ActivationFunctionType; AX = mybir.AxisListType

    caps = [int(x) for x in meta['caps']]
    slot_off = [int(x) for x in meta['slot_off']]
    TC = int(meta['tot_cols'])

    nc = bacc.Bacc("TRN2", target_bir_lowering=False, debug=False,
                   enable_asserts=True, num_devices=NCORES)

    t_xT = nc.dram_tensor("xT", [F_IN, N_PAD], bf16, kind="ExternalInput").ap()
    t_idx = nc.dram_tensor("idx", [P, TC * 8], i16, kind="ExternalInput").ap()
    t_m = nc.dram_tensor("m", [P, TC * 2], bf16, kind="ExternalInput").ap()
    t_W1 = nc.dram_tensor("W1full", [F_IN, 80], bf16, kind="ExternalInput").ap()
    t_W2 = nc.dram_tensor("W2full", [OUT1, 66], bf16, kind="ExternalInput").ap()
    t_out = nc.dram_tensor("out", [N_PAD, NLAB], f32, kind="ExternalOutput").ap()

    KCH = F_IN // P  # 4 k-chunks

    with tile.TileContext(nc) as tc:
        with tc.tile_pool(name="dram", bufs=1, space="DRAM") as dram, \
             tc.tile_pool(name="const", bufs=1) as cpool, \
             tc.tile_pool(name="work", bufs=4) as wpool, \
             tc.tile_pool(name="gath", bufs=4) as gpool, \
             tc.tile_pool(name="msgsp", bufs=3) as mpool, \
             tc.tile_pool(name="psum", bufs=2, space="PSUM") as pp, \
             tc.tile_pool(name="psum1", bufs=2, space="PSUM") as pp1:

            ident = cpool.tile([P, P], f32)
            make_identity(nc, ident[:])

            w1_sb = cpool.tile([P, KCH, 80], bf16)
            nc.sync.dma_start(out=w1_sb[:], in_=t_W1.rearrange("(k p) w -> p k w", p=P))
            w2_sb = cpool.tile([OUT1, 66], bf16)
            nc.sync.dma_start(out=w2_sb[:], in_=t_W2[:])
            m_sb = cpool.tile([P, TC, 2], bf16)
            nc.sync.dma_start(out=m_sb[:], in_=t_m[:].rearrange("p (c t) -> p c t", t=2))
            idx_sb = cpool.tile([P, TC * 8], i16)
            nc.sync.dma_start(out=idx_sb[:], in_=t_idx[:])

            ed1_all = cpool.tile([P, NBLK, H1], f32)
            ed2_all = cpool.tile([P, NBLK, 1], f32)

            # DRAM tables
            T1_loc = dram.tile([N_PAD, ROW], bf16)
            T2_loc = dram.tile([N_PAD, ROW], bf16)
            T1_full = dram.tile([N_TOT, ROW], bf16, addr_space="Shared")
            T2_full = dram.tile([N_TOT, ROW], bf16, addr_space="Shared")

            # ---------------- phase 1: transform layer 1 ----------------
            for b in range(NBLK):
                ps = pp.tile([P, 80], f32, tag="tf1", space="PSUM")
                if b % 4 == 0:
                    nb = min(4, NBLK - b)
                    xt4 = [None, None]
                    for kk in range(KCH // 2):
                        xt4[kk] = wpool.tile([P, 2, 4 * P], bf16, tag=f"xt{kk}",
                                             bufs=2, name=f"xt4_{kk}")
                        nc.sync.dma_start(
                            out=xt4[kk][:, :, 0:nb * P],
                            in_=t_xT[kk * 2 * P:(kk + 1) * 2 * P, b * P:(b + nb) * P]
                                .rearrange("(k p) n -> p k n", k=2))
                    blk_xt = xt4
                bo = (b % 4) * P
                for kk in range(KCH // 2):
                    for k2 in range(2):
                        k = kk * 2 + k2
                        nc.tensor.matmul(out=ps[:], lhsT=blk_xt[kk][:, k2, bo:bo + P],
                                         rhs=w1_sb[:, k, :], start=(k == 0), stop=(k == KCH - 1))
                row = wpool.tile([P, ROW], bf16, tag="trow")
                nc.vector.tensor_copy(out=row[:, 0:72], in_=ps[:, 0:72])
                if b == NBLK - 1:
                    # pad nodes (tail partitions) must have es = -1e30
                    npad = N_PAD - N_PC  # 22
                    nc.gpsimd.affine_select(
                        out=row[:, ES_OFF:ES_OFF + H1], in_=row[:, ES_OFF:ES_OFF + H1],
                        pattern=[[0, H1]], compare_op=mybir.AluOpType.is_ge,
                        fill=NEG_BIG, base=P - npad - 1, channel_multiplier=-1)
                nc.sync.dma_start(out=T1_loc[b * P:(b + 1) * P, :], in_=row[:])
                nc.vector.tensor_copy(out=ed1_all[:, b, :], in_=ps[:, 72:80])

            # ---------------- all-gather table 1 ----------------
            nc.gpsimd.collective_compute(
                "AllGather", mybir.AluOpType.bypass,
                replica_groups=[list(range(NCORES))],
                ins=[T1_loc[:].opt()], outs=[T1_full[:].opt()])

            # pair view of the full table: [NPAIR, 256]
            T1_pair = T1_full[:].rearrange("(q t) r -> q (t r)", t=2)
            T2_pair = T2_full[:].rearrange("(q t) r -> q (t r)", t=2)

            def edge_block(b, T_pair, ed_ap, H, layer):
                """Process dst-block b for one layer. Returns (numer, den) tiles."""
                cap = caps[b]; o0 = slot_off[b]
                ni = cap * P
                g = gpool.tile([P, caps[0], 256], bf16, tag="g")
                n_g = min((ni + MAX_GATHER - 1) // MAX_GATHER, 8)
                step = ((cap + n_g - 1) // n_g)
                c0 = 0
                while c0 < cap:
                    c1 = min(c0 + step, cap)
                    nc.gpsimd.dma_gather(
                        out_ap=g[:, c0:c1, :], in_ap=T_pair,
                        idxs_ap=idx_sb[:, (o0 + c0) * 8:(o0 + c1) * 8],
                        num_idxs=(c1 - c0) * P, num_idxs_reg=(c1 - c0) * P,
                        elem_size=256, single_packet=False)
                    c0 = c1
                mm = m_sb[:, o0:o0 + cap, :]      # [P, cap, 2] = [1-m | m]

                # both-halves attention: s = es{A,B} + ed; e = max(s, 0.2*s);
                # ex = exp(e); exps = ex * [1-m | m]
                es2 = g[:, 0:cap, :].rearrange("p j (t r) -> p j t r", t=2)[:, :, :, ES_OFF:ES_OFF + H]
                s = wpool.tile([P, caps[0], 2, H1], f32, tag="s", name="st")[:, 0:cap, :, 0:H]
                nc.vector.tensor_tensor(
                    out=s, in0=es2,
                    in1=ed_ap.unsqueeze(1).unsqueeze(1).broadcast_to([P, cap, 2, H]),
                    op=AL.add)
                e = wpool.tile([P, caps[0], 2, H1], f32, tag="e", name="et")[:, 0:cap, :, 0:H]
                nc.vector.scalar_tensor_tensor(out=e, in0=s, scalar=LRELU_SLOPE,
                                               in1=s, op0=AL.mult, op1=AL.max)
                ex = wpool.tile([P, caps[0], 2, H1], f32, tag="exx", name="exxt")[:, 0:cap, :, 0:H]
                nc.scalar.activation(out=ex, in_=e, func=AF.Exp)
                exps = wpool.tile([P, caps[0], 2, H1], bf16, tag="exps", name="expst")[:, 0:cap, :, 0:H]
                nc.vector.tensor_tensor(
                    out=exps, in0=ex,
                    in1=mm.unsqueeze(-1).broadcast_to([P, cap, 2, H]), op=AL.mult)

                # msgs[p, j, half, h, o] = Wh[p, j, half, h, o] * exps[p, j, half, h]
                OUTD = 64
                msgs = mpool.tile([P, caps[0], 2, OUTD], bf16, tag="msgs", name="msgst")[:, 0:cap, :, :]
                wh = g[:, 0:cap, :].rearrange("p j (t r) -> p j t r", t=2)[:, :, :, 0:OUTD]
                if H > 1:
                    wh5 = wh.rearrange("p j t (h o) -> p j t h o", h=H)
                    ex5 = exps.unsqueeze(-1).broadcast_to([P, cap, 2, H, OUTD // H])
                    nc.vector.tensor_tensor(out=msgs.rearrange("p j t (h o) -> p j t h o", h=H),
                                            in0=wh5, in1=ex5, op=AL.mult)
                else:
                    ex4 = exps.broadcast_to([P, cap, 2, OUTD])
                    nc.vector.tensor_tensor(out=msgs, in0=wh, in1=ex4, op=AL.mult)

                # tree-reduce over (j, half) with contiguous chunk adds
                tre = mpool.tile([P, caps[0], OUTD], f32, tag="tree", name="treet")
                nc.vector.tensor_tensor(out=tre[:, 0:cap, :], in0=msgs[:, :, 0, :],
                                        in1=msgs[:, :, 1, :], op=AL.add)
                cur = cap
                while cur > 1:
                    mhalf = cur // 2
                    nc.vector.tensor_tensor(out=tre[:, 0:mhalf, :], in0=tre[:, 0:mhalf, :],
                                            in1=tre[:, cur - mhalf:cur, :], op=AL.add)
                    cur -= mhalf
                numer = tre[:, 0, :]
                den = wpool.tile([P, H1], f32, tag="den", name="dent")[:, 0:H]
                nc.vector.tensor_reduce(out=den, in_=exps.rearrange("p j t h -> p h (j t)"),
                                        axis=AX.X, op=AL.add)
                nc.vector.tensor_scalar_add(out=den, in0=den, scalar1=1e-10)
                rec = wpool.tile([P, H1], f32, tag="rec", name="rect")[:, 0:H]
                nc.vector.reciprocal(out=rec, in_=den)
                hpre = wpool.tile([P, OUTD], f32, tag="hpre")
                if H > 1:
                    nc.vector.tensor_tensor(
                        out=hpre[:].rearrange("p (h o) -> p h o", h=H),
                        in0=numer.rearrange("p (h o) -> p h o", h=H),
                        in1=rec.unsqueeze(-1).broadcast_to([P, H, OUTD // H]), op=AL.mult)
                else:
                    nc.vector.tensor_tensor(out=hpre[:], in0=numer,
                                            in1=rec.broadcast_to([P, OUTD]), op=AL.mult)
                return hpre

            # ---------------- phase 2: layer-1 edges + layer-2 transform ----------------
            for b in range(NBLK):
                hpre = edge_block(b, T1_pair, ed1_all[:, b, :], H1, 1)
                # ELU: h = relu(x) + min(exp(x),1) - 1
                ex_h = wpool.tile([P, OUT1], f32, tag="eluex")
                nc.scalar.activation(out=ex_h[:], in_=hpre[:], func=AF.Exp)
                r_h = wpool.tile([P, OUT1], f32, tag="elur")
                nc.vector.tensor_scalar_max(out=r_h[:], in0=hpre[:], scalar1=0.0)
                h = wpool.tile([P, OUT1], f32, tag="hfin")
                nc.vector.scalar_tensor_tensor(out=h[:], in0=ex_h[:], scalar=1.0,
                                               in1=r_h[:], op0=AL.min, op1=AL.add)
                nc.vector.tensor_scalar_add(out=h[:], in0=h[:], scalar1=-1.0)
                # transpose h -> [64, 128]
                hT_ps = pp1.tile([OUT1, P], f32, tag="hT", space="PSUM")
                nc.tensor.transpose(out=hT_ps[:], in_=h[:], identity=ident[:])
                hT = wpool.tile([OUT1, P], bf16, tag="hTb")
                nc.vector.tensor_copy(out=hT[:], in_=hT_ps[:])
                # layer-2 transform
                ps2 = pp.tile([P, 66], f32, tag="tf2", space="PSUM")
                nc.tensor.matmul(out=ps2[:], lhsT=hT[:], rhs=w2_sb[:], start=True, stop=True)
                row2 = wpool.tile([P, ROW], bf16, tag="trow")
                nc.vector.tensor_copy(out=row2[:, 0:65], in_=ps2[:, 0:65])
                if b == NBLK - 1:
                    npad = N_PAD - N_PC
                    nc.gpsimd.affine_select(
                        out=row2[:, ES_OFF:ES_OFF + 1], in_=row2[:, ES_OFF:ES_OFF + 1],
                        pattern=[[0, 1]], compare_op=mybir.AluOpType.is_ge,
                        fill=NEG_BIG, base=P - npad - 1, channel_multiplier=-1)
                nc.sync.dma_start(out=T2_loc[b * P:(b + 1) * P, :], in_=row2[:])
                nc.vector.tensor_copy(out=ed2_all[:, b, :], in_=ps2[:, 65:66])

            # ---------------- all-gather table 2 ----------------
            nc.gpsimd.collective_compute(
                "AllGather", mybir.AluOpType.bypass,
                replica_groups=[list(range(NCORES))],
                ins=[T2_loc[:].opt()], outs=[T2_full[:].opt()])

            # ---------------- phase 3: layer-2 edges + softmax ----------------
            for b in range(NBLK):
                opre = edge_block(b, T2_pair, ed2_all[:, b, :], H2, 2)
                rm = wpool.tile([P, 1], f32, tag="rm")
                nc.vector.tensor_reduce(out=rm[:], in_=opre[:], axis=AX.X,
                                        op=AL.max, negate=True)
                z = wpool.tile([P, NLAB], f32, tag="z")
                zsum = wpool.tile([P, 1], f32, tag="zsum")
                nc.scalar.activation(out=z[:], in_=opre[:], func=AF.Exp,
                                     bias=rm[:], accum_out=zsum[:])
                recs = wpool.tile([P, 1], f32, tag="recs")
                nc.vector.reciprocal(out=recs[:], in_=zsum[:])
                fin = wpool.tile([P, NLAB], f32, tag="fin")
                nc.vector.tensor_tensor(out=fin[:], in0=z[:],
                                        in1=recs[:].broadcast_to([P, NLAB]), op=AL.mult)
                nc.sync.dma_start(out=t_out[b * P:(b + 1) * P, :], in_=fin[:])

    nc.compile()
    return nc


def _install_ntff_shim():
    """antenv.axon_hooks is absent in this image; register the NTFF profile
    hook so trace=True can capture exec times. No-op if already present."""
    import types
    try:
        import antenv.axon_hooks  # noqa: F401
        return
    except ImportError:
        pass
    try:
        import antenv
        from trn_agent_boot.trn_boot import _ntff_profile_via_ctypes
        mod = types.ModuleType("antenv.axon_hooks")
        mod._hook = _ntff_profile_via_ctypes('/opt/axon/libaxon_pjrt.so')
        mod.set_axon_ntff_profile_hook = lambda h: setattr(mod, "_hook", h)
        mod.get_axon_ntff_profile_hook = lambda: mod._hook
        sys.modules["antenv.axon_hooks"] = mod
        antenv.axon_hooks = mod
    except Exception:
        pass


def kernel(inputs, W1, a1_src, a1_dst, W2, a2_src, a2_dst, src, dst):
    from concourse import bass_utils
    if int(os.environ.get("GAT_TRACE", "0")):
        _install_ntff_shim()
    meta, per_core, shared = host_prep(inputs, W1, a1_src, a1_dst, W2,
                                       a2_src, a2_dst, src, dst)
    nc = build_kernel(meta)
    in_maps = []
    for c in range(NCORES):
        in_maps.append(dict(
            xT=per_core['xT'][c], idx=per_core['idx'][c], m=per_core['m'][c],
            W1full=shared['W1full'], W2full=shared['W2full']))
    res = bass_utils.run_bass_kernel_spmd(
        nc, in_maps, core_ids=list(range(NCORES)),
        trace=bool(int(os.environ.get("GAT_TRACE", "0"))),
        trace_cores=list(range(NCORES)) if int(os.environ.get("GAT_TRACE", "0")) else None)
    kernel.last_exec_time_ns = res.exec_time_ns
    out = np.zeros((N, NLAB), dtype=np.float32)
    for c in range(NCORES):
        o = res.results[c]["out"]
        perm = meta['perms'][c]
        real = perm >= 0
        out[perm[real]] = o[real]
    return out


def mirror(inputs, W1, a1_src, a1_dst, W2, a2_src, a2_dst, src, dst):
    """Numpy mirror of the kernel's exact dataflow (incl. bf16 rounding of
    tables) for layout validation without hardware."""
    meta, per_core, shared = host_prep(inputs, W1, a1_src, a1_dst, W2,
                                       a2_src, a2_dst, src, dst)
    caps = meta['caps']; slot_off = meta['slot_off']; TC = meta['tot_cols']
    W1full = shared['W1full'].astype(np.float32)
    W2full = shared['W2full'].astype(np.float32)
    out = np.zeros((N, NLAB), dtype=np.float32)

    # build tables per core, then allgather
    T1 = np.zeros((N_TOT, ROW), dtype=np.float32)
    ed1 = np.zeros((NCORES, N_PAD, H1), dtype=np.float32)
    for c in range(NCORES):
        xT = per_core['xT'][c].astype(np.float32)
        t = xT.T @ W1full                       # [N_PAD, 80]
        rows = np.zeros((N_PAD, ROW), np.float32)
        rows[:, 0:72] = t[:, 0:72]
        rows[N_PC:, ES_OFF:ES_OFF + H1] = NEG_BIG
        T1[c * N_PAD:(c + 1) * N_PAD] = _bf16(rows).astype(np.float32)
        ed1[c] = t[:, 72:80]

    def edge_phase(c, T, ed, H):
        Tp = T.reshape(NPAIR, 256)
        idx = per_core['idx'][c]
        m = per_core['m'][c].astype(np.float32)
        res = np.zeros((N_PAD, OUT1), np.float32)
        for b in range(NBLK):
            cap = caps[b]; o0 = slot_off[b]
            # unwrap idx: stored wrapped per block
            wr = idx[:16, o0 * 8:(o0 + cap) * 8]
            flat = wr.T.reshape(-1)             # undo .reshape(-1,16).T
            g = Tp[flat.astype(np.int64)].reshape(cap, P, 256).transpose(1, 0, 2)
            mm = m[:, o0:o0 + cap]
            esA = g[:, :, ES_OFF:ES_OFF + H]
            esB = g[:, :, 128 + ES_OFF:128 + ES_OFF + H]
            es = esA + mm[:, :, None] * (esB - esA)
            s = es + ed[b * P:(b + 1) * P].reshape(P, 1, H)
            e = np.where(s > 0, s, LRELU_SLOPE * s)
            ex = np.exp(e)
            exB = ex * mm[:, :, None]; exA = ex - exB
            whA = g[:, :, 0:64]; whB = g[:, :, 128:192]
            if H > 1:
                o = OUT1 // H
                msA = whA.reshape(P, cap, H, o) * exA[:, :, :, None]
                msB = whB.reshape(P, cap, H, o) * exB[:, :, :, None]
                numer = (msA + msB).sum(axis=1).reshape(P, OUT1)
                den = (exA + exB).sum(axis=1)
                hpre = (numer.reshape(P, H, o) / (den[:, :, None] + 1e-10)).reshape(P, OUT1)
            else:
                msA = whA * exA; msB = whB * exB
                numer = (msA + msB).sum(axis=1)
                den = (exA + exB).sum(axis=1)
                hpre = numer / (den + 1e-10)
            res[b * P:(b + 1) * P] = hpre
        return res

    T2 = np.zeros((N_TOT, ROW), dtype=np.float32)
    ed2 = np.zeros((NCORES, N_PAD, 1), dtype=np.float32)
    h_all = {}
    for c in range(NCORES):
        hpre = edge_phase(c, T1, ed1[c], H1)
        h = np.maximum(hpre, 0) + np.minimum(np.exp(hpre), 1.0) - 1.0
        h_all[c] = h
        t2 = _bf16(h).astype(np.float32) @ W2full
        rows = np.zeros((N_PAD, ROW), np.float32)
        rows[:, 0:65] = t2[:, 0:65]
        rows[N_PC:, ES_OFF:ES_OFF + 1] = NEG_BIG
        T2[c * N_PAD:(c + 1) * N_PAD] = _bf16(rows).astype(np.float32)
        ed2[c] = t2[:, 65:66]

    for c in range(NCORES):
        opre = edge_phase(c, T2, ed2[c], H2)[:, 0:NLAB]
        z = np.exp(opre - opre.max(axis=1, keepdims=True))
        fin = z / z.sum(axis=1, keepdims=True)
        perm = meta['perms'][c]; real = perm >= 0
        out[perm[real]] = fin[real]
    return out
